# revision 2
# baseline (speedup 1.0000x reference)
"""Multi-head self-attention (B=4, S=2048, D=1024, H=16, causal + RoPE) on 8
Trainium2 NeuronCores.  v2: PE tile-pair concurrency + causal trims.

Sharding: core c = (batch b = c // 2, head-group hg = c % 2).  Host sums the
two partial outputs per batch.

Device program (fp16 matmul operands, fp32 PSUM accumulation):
  - Scores emitted as adjacent row-tiled pairs (h0 rows 0-63, h1 rows 64-127,
    different PSUM banks) -> both heads stream concurrently (~2x).
  - Wo emitted as adjacent (h, half)-alternating pairs into two PSUM banks
    po_a (out cols 0-511) / po_b (cols 512-1023): concurrent, and no
    post-merge needed (each bank is a complete output half).
  - Diagonal key blocks: scores/attn@V/mask restricted to the valid query
    range (q >= 128*j within the chunk).
  - One exp per key-block-pair over [128, 2048] PSUM -> fewer ScalarE calls.
  - ctxu evacuation carries the Z row (f32, 65 partitions); Z reshape via
    DRAM roundtrip to [64, 16], reciprocal 64 lanes wide, broadcast-read DMA.
  - proj/Wo chains woven between attention kbp iterations to fill the PE
    while ScalarE exp paces the attention pipeline.
"""

import os
import numpy as np

K2_WEAVE = os.environ.get("K2_WEAVE", "1") == "1"
K2_QSPLIT = os.environ.get("K2_QSPLIT", "1") == "1"
K2_EXP2 = os.environ.get("K2_EXP2", "1") == "1"    # single [128,2048] exp call
K2_TRIM = os.environ.get("K2_TRIM", "1") == "1"    # attnV+mask q-trim on diagonal
K2_WOPAIR = os.environ.get("K2_WOPAIR", "1") == "1"  # Wo alternating po_a/po_b

D_MODEL = 1024
NUM_HEADS = 16
D_K = 64
ROPE_THETA = 10000.0
B = 4
S = 2048
N_CORES = 8

HG_FEATS = 512          # features per core (8 heads)
FT = HG_FEATS // 128    # head pairs per core
KT = D_MODEL // 128     # contraction tiles for the projections

MM_DTYPE = "f16"
_PROGRAM_CACHE = {}


def _build_program(use_rope: bool, s: int = S):
    import concourse.tile as tile
    from concourse import bacc, mybir
    from contextlib import ExitStack

    f32 = mybir.dt.float32
    f16 = mybir.dt.float16
    mmdt = f16
    FP = mybir.ActivationFunctionType

    SC = s // 512           # 512-wide seq chunks
    ST = s // 128           # 128-wide seq tiles
    PAIRSWAP = [i ^ 1 for i in range(32)]

    nc = bacc.Bacc("TRN2", target_bir_lowering=False, debug=False)

    xT = nc.dram_tensor("xT", [D_MODEL, s], mmdt, kind="ExternalInput")
    wqT = nc.dram_tensor("wqT", [D_MODEL, HG_FEATS], mmdt, kind="ExternalInput")
    wkT = nc.dram_tensor("wkT", [D_MODEL, HG_FEATS], mmdt, kind="ExternalInput")
    wvT = nc.dram_tensor("wvT", [D_MODEL, HG_FEATS], mmdt, kind="ExternalInput")
    woT = nc.dram_tensor("woT", [HG_FEATS, D_MODEL], mmdt, kind="ExternalInput")
    if use_rope:
        cosT = nc.dram_tensor("cosT", [128, s], f16, kind="ExternalInput")
        sinT = nc.dram_tensor("sinT", [128, s], f16, kind="ExternalInput")
    if K2_WOPAIR:
        out_a = nc.dram_tensor("out_a", [s, D_MODEL], f16, kind="ExternalOutput")
        out_b = nc.dram_tensor("out_b", [s, D_MODEL], f16, kind="ExternalOutput")
    else:
        out = nc.dram_tensor("out", [s, D_MODEL], f32, kind="ExternalOutput")

    with tile.TileContext(nc) as tc, ExitStack() as ctx:
        singles = ctx.enter_context(tc.tile_pool(name="singles", bufs=1))
        stripes = ctx.enter_context(tc.tile_pool(name="stripes", bufs=2))
        tmppool = ctx.enter_context(tc.tile_pool(name="tmppool", bufs=2))
        exppool = ctx.enter_context(tc.tile_pool(name="exppool", bufs=3))
        ctxpool = ctx.enter_context(tc.tile_pool(name="ctxpool", bufs=3))
        smallp = ctx.enter_context(tc.tile_pool(name="smallp", bufs=3))
        outpool = ctx.enter_context(tc.tile_pool(name="outpool", bufs=4))
        dramp = ctx.enter_context(tc.tile_pool(name="dramp", bufs=1, space="DRAM"))
        psum = ctx.enter_context(tc.tile_pool(name="psum", bufs=1, space="PSUM"))

        # ---- persistent tiles -------------------------------------------
        wq_full = singles.tile([128, KT, HG_FEATS], mmdt, tag="wqf")
        wk_full = singles.tile([128, KT, HG_FEATS], mmdt, tag="wkf")
        wv_full = singles.tile([128, KT, HG_FEATS], mmdt, tag="wvf")
        wo_sb = singles.tile([128, FT, D_MODEL], mmdt, tag="wo")

        # Initial loads split across HWDGE queues: sync carries what the
        # first V matmuls need (wv + stripe 0); vector carries wq/wk; scalar
        # carries wo (+ rope tables).
        nc.sync.dma_start(
            out=wv_full[:, 0 : KT // 2, :],
            in_=wvT.ap()[0 : D_MODEL // 2, :].rearrange("(k p) f -> p k f", p=128),
        )
        sh_first = stripes.tile([128, KT, 512], mmdt, tag="stripe", name="sh0")
        for w in range(2):
            ks = slice(w * KT // 2, (w + 1) * KT // 2)
            nc.sync.dma_start(
                out=sh_first[:, ks, :],
                in_=xT.ap()[(ks.start * 128) : (ks.stop * 128), 0:512].rearrange(
                    "(k p) s -> p k s", p=128
                ),
            )
        nc.sync.dma_start(
            out=wv_full[:, KT // 2 : KT, :],
            in_=wvT.ap()[D_MODEL // 2 :, :].rearrange("(k p) f -> p k f", p=128),
        )
        qldma = nc.scalar if K2_QSPLIT else nc.sync
        for wsb, wdr in ((wq_full, wqT), (wk_full, wkT)):
            qldma.dma_start(
                out=wsb, in_=wdr.ap().rearrange("(k p) f -> p k f", p=128)
            )
        if use_rope:
            cos_sb = singles.tile([128, s], f16, tag="cos")
            sin_sb = singles.tile([128, s], f16, tag="sin")
            qldma.dma_start(out=cos_sb, in_=cosT.ap())
            qldma.dma_start(out=sin_sb, in_=sinT.ap())
        _wo_load = [
            lambda: qldma.dma_start(
                out=wo_sb, in_=woT.ap().rearrange("(f p) o -> p f o", p=128)
            )
        ]

        # V resident in SBUF: per (kb, hp) cols [V_h0(64) | 1 | pad | V_h1(64) | 1 | pad]
        vres = singles.tile([128, ST, FT, 132], mmdt, tag="vres")
        nc.vector.memset(
            vres[:].rearrange("p st hp (h c) -> p (st hp h) c", c=66)[:, :, 64:65],
            1.0,
        )
        # Z rows roundtrip through DRAM: [1, 1024] -> [64, 16] for a 64-lane
        # reciprocal, then 1/Z broadcast back across 64 partitions.
        ztmp = dramp.tile([FT, SC, 1, 1024], f32, tag="ztmp")
        ztmp2 = dramp.tile([FT, SC, 1, 1024], f16, tag="ztmp2")
        QTs = [
            singles.tile([128, s], mmdt, tag=f"QT{hp}", name=f"QT{hp}")
            for hp in range(FT)
        ]
        KTs = [
            singles.tile([128, s], mmdt, tag=f"KT{hp}", name=f"KT{hp}")
            for hp in range(FT)
        ]

        def load_stripe(sc):
            sh = stripes.tile([128, KT, 512], mmdt, tag="stripe", name=f"sh{sc}")
            nc.sync.dma_start(
                out=sh,
                in_=xT.ap()[:, sc * 512 : (sc + 1) * 512].rearrange(
                    "(k p) s -> p k s", p=128
                ),
            )
            return sh

        # ---- work chains (each closure emits ~1-2us of PE work) ----------
        def vproj_chain(sc, sh, j):
            def emit():
                st = sc * 4 + j
                pv = psum.tile([128, 512], f32, tag="pqk", bufs=2, name="pv")
                for kt in range(KT):
                    nc.tensor.matmul(
                        pv,
                        lhsT=sh[:, kt, j * 128 : (j + 1) * 128],
                        rhs=wv_full[:, kt, :],
                        start=(kt == 0),
                        stop=(kt == KT - 1),
                    )
                # scatter into vres: dest inner 64 f16 contiguous, 132-col
                # blocks keep h-offsets 4B-aligned (DVE evacuation)
                nc.vector.tensor_copy(
                    vres[:, st].rearrange("p hp (h c) -> p (hp h) c", c=66)[
                        :, :, 0:64
                    ],
                    pv[:].rearrange("p (x c) -> p x c", c=64),
                )
            return emit

        def qkproj_chain(sc, sh, hp, which):
            def emit():
                w_sb = wq_full if which == 0 else wk_full
                dst = QTs[hp] if which == 0 else KTs[hp]
                pq = psum.tile([128, 512], f32, tag="pqk", bufs=2, name="pq")
                for kt in range(KT):
                    nc.tensor.matmul(
                        pq,
                        lhsT=w_sb[:, kt, hp * 128 : (hp + 1) * 128],
                        rhs=sh[:, kt, :],
                        start=(kt == 0),
                        stop=(kt == KT - 1),
                    )
                dcol = dst[:, sc * 512 : (sc + 1) * 512]
                if use_rope:
                    ccol = cos_sb[:, sc * 512 : (sc + 1) * 512]
                    scol = sin_sb[:, sc * 512 : (sc + 1) * 512]
                    qsb = tmppool.tile([128, 512], f16, tag="qsb")
                    qcos = tmppool.tile([128, 512], f16, tag="qcos")
                    rot = tmppool.tile([128, 512], f16, tag="rot")
                    nc.scalar.activation(qsb, pq, FP.Copy)
                    nc.vector.stream_shuffle(rot, qsb, PAIRSWAP)
                    nc.vector.tensor_mul(qcos, qsb, ccol)
                    nc.vector.tensor_mul(rot, rot, scol)
                    nc.vector.tensor_add(dcol, qcos, rot)
                else:
                    nc.vector.tensor_copy(dcol, pq)
            return emit

        def wo_chain_base(qc, ctxn, j):
            def emit():
                st = qc * 4 + j
                osb = outpool.tile([128, 1024], f32, tag="osb")
                for half in range(2):
                    po = [
                        psum.tile([128, 512], f32, tag="pqk", bufs=2, name=f"po{h}")
                        for h in range(2)
                    ]
                    for hp in range(FT):
                        for h in range(2):
                            nc.tensor.matmul(
                                po[h],
                                lhsT=ctxn[hp][
                                    64 * h : 64 * h + 64, j * 128 : (j + 1) * 128
                                ],
                                rhs=wo_sb[
                                    64 * h : 64 * h + 64,
                                    hp,
                                    half * 512 : (half + 1) * 512,
                                ],
                                start=(hp == 0),
                                stop=(hp == FT - 1),
                                skip_group_check=True,
                            )
                    oh = osb[:, half * 512 : (half + 1) * 512]
                    nc.scalar.activation(oh, po[0], FP.Copy)
                    nc.vector.scalar_tensor_tensor(
                        out=oh,
                        in0=po[1],
                        scalar=1.0,
                        in1=oh,
                        op0=mybir.AluOpType.mult,
                        op1=mybir.AluOpType.add,
                    )
                nc.sync.dma_start(
                    out=out.ap()[st * 128 : (st + 1) * 128, :], in_=osb
                )
            return emit

        def wo_chain(qc, ctxn, j, epilogue=False):
            if not K2_WOPAIR:
                return wo_chain_base(qc, ctxn, j)

            def emit():
                st = qc * 4 + j
                # 4 chains, each with a consistent PE row group:
                #   h0 chains (rows 0-63)  -> osb_a -> out_a
                #   h1 chains (rows 64-127) -> osb_b -> out_b
                # (h0, half) and (h1, half) emitted adjacently -> concurrent.
                osb_a = outpool.tile([128, 1024], f16, tag="osba")
                osb_b = outpool.tile([128, 1024], f16, tag="osbb")
                for half in range(2):
                    if epilogue and (2 * j + half) % 2 == 1:
                        pop = psum.tile(
                            [128, 1024], f32, tag="oacc", bufs=1, name="pop"
                        )
                        po0 = pop[:, 0:512]
                        po1 = pop[:, 512:1024]
                    else:
                        po0 = psum.tile([128, 512], f32, tag="pqk", bufs=2, name="po0")
                        po1 = psum.tile([128, 512], f32, tag="pqk", bufs=2, name="po1")
                    for hp in range(FT):
                        for h, po in ((0, po0), (1, po1)):
                            nc.tensor.matmul(
                                po,
                                lhsT=ctxn[hp][
                                    64 * h : 64 * h + 64, j * 128 : (j + 1) * 128
                                ],
                                rhs=wo_sb[
                                    64 * h : 64 * h + 64,
                                    hp,
                                    half * 512 : (half + 1) * 512,
                                ],
                                start=(hp == 0),
                                stop=(hp == FT - 1),
                                skip_group_check=True,
                            )
                    # evacuate: one on ScalarE, one on DVE (engine balance)
                    nc.scalar.activation(
                        osb_a[:, half * 512 : (half + 1) * 512], po0, FP.Copy
                    )
                    nc.vector.tensor_copy(
                        osb_b[:, half * 512 : (half + 1) * 512], po1
                    )
                nc.sync.dma_start(
                    out=out_a.ap()[st * 128 : (st + 1) * 128, :], in_=osb_a
                )
                nc.sync.dma_start(
                    out=out_b.ap()[st * 128 : (st + 1) * 128, :], in_=osb_b
                )
            return emit

        # ---- attention ---------------------------------------------------
        def do_attn(qc, work):
            nkb = 4 * qc + 4
            ctxn = [
                ctxpool.tile([128, 512], f16, tag=f"ctxn{hp}", name=f"ctxn{hp}")
                for hp in range(FT)
            ]
            for hp in range(FT):
                QT = QTs[hp]
                KTt = KTs[hp]
                oacc = psum.tile([128, 1024], f32, tag="oacc", bufs=1, name="oacc")
                def emit_attnv(kb, expt):
                    j = kb - 4 * qc
                    q0 = 128 * j if (j >= 0 and K2_TRIM) else 0
                    for h in range(2):
                        nc.tensor.matmul(
                            oacc[0:65, 512 * h + q0 : 512 * h + 512],
                            lhsT=vres[:, kb, hp, 66 * h : 66 * h + 65],
                            rhs=expt[:, 512 * h + q0 : 512 * h + 512],
                            start=(kb == 0),
                            stop=(kb == nkb - 1),
                            skip_group_check=True,
                        )

                prev = None
                for kb in range(nkb):
                    if kb % 2 == 0:
                        _drain_due(work, (qc, hp, kb // 2))
                    # scores [128, 1024]: [h0 | h1], double-buffered
                    scr = psum.tile(
                        [128, 1024], f32, tag="scores", bufs=2, name="scores"
                    )
                    for h in range(2):
                        nc.tensor.matmul(
                            scr[:, 512 * h : 512 * h + 512],
                            lhsT=KTt[
                                64 * h : 64 * h + 64, kb * 128 : (kb + 1) * 128
                            ],
                            rhs=QT[
                                64 * h : 64 * h + 64,
                                qc * 512 : (qc + 1) * 512,
                            ],
                            start=True,
                            stop=True,
                            skip_group_check=True,
                        )
                    expt = exppool.tile([128, 1024], mmdt, tag="expt")
                    nc.scalar.activation(expt, scr, FP.Exp, scale=0.125)
                    j = kb - 4 * qc
                    if j >= 0:  # diagonal tile: causal mask post-exp
                        q0 = 128 * j if K2_TRIM else 0
                        for h in range(2):
                            sl = expt[:, 512 * h + q0 : 512 * h + 512]
                            nc.gpsimd.affine_select(
                                out=sl,
                                in_=sl,
                                compare_op=mybir.AluOpType.is_ge,
                                fill=0.0,
                                base=-(128 * j - q0),
                                pattern=[[1, 512 - q0]],
                                channel_multiplier=-1,
                            )
                    # previous kb's attn@V (exp/mask latency covered)
                    if prev is not None:
                        emit_attnv(*prev)
                    prev = (kb, expt)
                emit_attnv(*prev)
                # ---- evacuate ctx+Z, normalize -------------------------
                ctxu = smallp.tile([128, 1024], f32, tag="ctxu")
                nc.vector.tensor_copy(ctxu[0:65, :], oacc[0:65, :])
                dmae = nc.sync
                dmae.dma_start(out=ztmp[hp, qc], in_=ctxu[64:65, :])
                zT = smallp.tile([64, 16], f32, tag="zT")
                dmae.dma_start(
                    out=zT, in_=ztmp[hp, qc, 0].rearrange("(p g) -> p g", p=64)
                )
                zinvT = smallp.tile([64, 16], f32, tag="zinvT")
                nc.vector.reciprocal(zinvT, zT)
                zinvT16 = smallp.tile([64, 16], f16, tag="zinvT16")
                nc.vector.tensor_copy(zinvT16, zinvT)
                dmae.dma_start(
                    out=ztmp2[hp, qc, 0].rearrange("(p g) -> p g", p=64),
                    in_=zinvT16,
                )
                zbc_sb = smallp.tile([128, 1024], f16, tag="zbcsb")
                dmae.dma_start(
                    out=zbc_sb[0:64, :],
                    in_=ztmp2[hp, qc].broadcast_to([64, 1024]),
                )
                for h in range(2):
                    nc.vector.scalar_tensor_tensor(
                        out=ctxn[hp][64 * h : 64 * h + 64, :],
                        in0=ctxu[0:64, 512 * h : 512 * h + 512],
                        scalar=1.0,
                        in1=zbc_sb[0:64, 512 * h : 512 * h + 512],
                        op0=mybir.AluOpType.mult,
                        op1=mybir.AluOpType.mult,
                    )
            return ctxn

        # ---- deadline-scheduled emission ---------------------------------
        # Every chain gets an emission deadline (qc, hp, kbp): it is emitted
        # just before that attention step.  Deadlines place each chain where
        # the PE needs fill (late chunks have more exp time to cover), always
        # at-or-before the step that first reads the chain's outputs.
        work = []

        def _drain_due(work, step):
            i = 0
            while i < len(work):
                dl, em = work[i]
                if dl <= step:
                    em()
                    work.pop(i)
                else:
                    i += 1

        # proj(0): V j0/j1 + QK hp0 land in the prologue (deadline (0,0,0));
        # the rest weave into attn(0) at their first-read step.
        PROJ_DL = {
            0: {("v", 0): (0, 0, 0), ("v", 1): (0, 0, 0),
                ("v", 2): (0, 0, 1), ("v", 3): (0, 0, 1),
                ("qk", 0, 0): (0, 0, 0), ("qk", 0, 1): (0, 0, 0),
                ("qk", 1, 0): (0, 1, 0), ("qk", 1, 1): (0, 1, 0),
                ("qk", 2, 0): (0, 2, 0), ("qk", 2, 1): (0, 2, 0),
                ("qk", 3, 0): (0, 3, 0), ("qk", 3, 1): (0, 3, 0)},
            # proj(1): V + QK(hp0) inside attn(0); the rest early in attn(1)
            1: {("v", 0): (0, 0, 1), ("v", 1): (0, 1, 1),
                ("v", 2): (0, 2, 1), ("v", 3): (0, 3, 1),
                ("qk", 0, 0): (0, 1, 1), ("qk", 0, 1): (0, 2, 1),
                ("qk", 1, 0): (1, 0, 0), ("qk", 1, 1): (1, 0, 1),
                ("qk", 2, 0): (1, 1, 0), ("qk", 2, 1): (1, 1, 1),
                ("qk", 3, 0): (1, 2, 0), ("qk", 3, 1): (1, 2, 1)},
            # proj(2): spread across attn(1)
            2: {("v", 0): (1, 0, 1), ("v", 1): (1, 1, 1),
                ("v", 2): (1, 2, 1), ("v", 3): (1, 3, 1),
                ("qk", 0, 0): (1, 0, 2), ("qk", 0, 1): (1, 0, 3),
                ("qk", 1, 0): (1, 1, 2), ("qk", 1, 1): (1, 1, 3),
                ("qk", 2, 0): (1, 2, 2), ("qk", 2, 1): (1, 2, 3),
                ("qk", 3, 0): (1, 3, 2), ("qk", 3, 1): (1, 3, 3)},
            # proj(3): V + QK(hp0) spread across attn(2); QK(hp1-3) spill
            # into attn(3) just before each hp needs them
            3: {("v", 0): (2, 0, 2), ("v", 1): (2, 0, 5),
                ("v", 2): (2, 1, 1), ("v", 3): (2, 1, 4),
                ("qk", 0, 0): (2, 0, 0), ("qk", 0, 1): (2, 2, 0),
                ("qk", 1, 0): (3, 0, 1), ("qk", 1, 1): (3, 0, 4),
                ("qk", 2, 0): (3, 1, 1), ("qk", 2, 1): (3, 1, 5),
                ("qk", 3, 0): (3, 2, 2), ("qk", 3, 1): (3, 2, 4)},
        }
        # wo(sc) spread: wo(0) into attn(2); wo(1)/wo(2) into attn(3)
        WO_DL = {
            0: [(2, 2, 2), (2, 2, 5), (2, 3, 1), (2, 3, 3)],
            1: [(3, 0, 6), (3, 1, 3), (3, 1, 7), (3, 2, 6)],
            2: [(3, 3, 1), (3, 3, 3), (3, 3, 5), (3, 3, 7)],
        }

        def add_proj_work(sc, sh):
            dls = PROJ_DL[sc]
            for j in range(4):
                work.append((dls[("v", j)], vproj_chain(sc, sh, j)))
            for hp in range(FT):
                for w in (0, 1):
                    work.append((dls[("qk", hp, w)], qkproj_chain(sc, sh, hp, w)))

        add_proj_work(0, sh_first)
        work.append(((0, 2, 0), _wo_load[0]))
        ctxs = {}
        for sc in range(SC):
            if sc + 1 < SC:
                sh = load_stripe(sc + 1)
                add_proj_work(sc + 1, sh)
            if sc - 1 >= 0 and sc - 1 in WO_DL:
                prev_ctx = ctxs[sc - 1]
                for j in range(4):
                    work.append((WO_DL[sc - 1][j], wo_chain(sc - 1, prev_ctx, j)))
            ctxs[sc] = do_attn(sc, work)
        for dl, em in work:  # anything left (defensive)
            em()
        work.clear()
        last_ctx = ctxs[SC - 1]
        for em in [wo_chain(SC - 1, last_ctx, j, epilogue=True) for j in range(4)]:
            em()

    nc.compile()
    return nc


def _round_f32r(a):
    """Round fp32 array to the PE's FP32R format (RNE at 12 low mantissa bits)."""
    u = np.ascontiguousarray(a, np.float32).view(np.uint32).astype(np.uint64)
    low = u & 0xFFF
    up = (low > 0x800) | ((low == 0x800) & (((u >> 12) & 1) == 1))
    r = (u & ~np.uint64(0xFFF)) + np.where(up, 0x1000, 0)
    return r.astype(np.uint32).view(np.float32)


def _to_f16(a):
    return np.ascontiguousarray(a, np.float16)


def _rope_tables(s: int):
    inv_freq = 1.0 / (ROPE_THETA ** (np.arange(0, D_K, 2, dtype=np.float64) / D_K))
    angles = np.arange(s, dtype=np.float64)[:, None] * inv_freq[None, :]  # [s, 32]
    cos = np.cos(angles).astype(np.float32)  # [s, 32]
    sin = np.sin(angles).astype(np.float32)
    cosT = np.empty((D_K, s), np.float32)
    sinT = np.empty((D_K, s), np.float32)
    cosT[0::2] = cos.T
    cosT[1::2] = cos.T
    sinT[0::2] = -sin.T
    sinT[1::2] = sin.T
    return (
        np.ascontiguousarray(np.vstack([cosT, cosT])).astype(np.float16),
        np.ascontiguousarray(np.vstack([sinT, sinT])).astype(np.float16),
    )


def kernel(x, Wq, Wk, Wv, Wo, use_rope):
    from concourse.bass_utils import run_bass_kernel_spmd

    x = np.asarray(x, dtype=np.float32)
    ur = bool(int(np.asarray(use_rope)))
    key = (ur, S)
    if key not in _PROGRAM_CACHE:
        _PROGRAM_CACHE[key] = _build_program(ur, S)
    nc = _PROGRAM_CACHE[key]

    if ur:
        cosT, sinT = _rope_tables(S)

    in_maps = []
    for c in range(N_CORES):
        b, hg = c // 2, c % 2
        sl = slice(hg * HG_FEATS, (hg + 1) * HG_FEATS)
        cv = _round_f32r if MM_DTYPE == "f32r" else _to_f16
        m = {
            "xT": cv(x[b].T),
            "wqT": cv(np.asarray(Wq, np.float32)[sl, :].T),
            "wkT": cv(np.asarray(Wk, np.float32)[sl, :].T),
            "wvT": cv(np.asarray(Wv, np.float32)[sl, :].T),
            "woT": cv(np.asarray(Wo, np.float32)[:, sl].T),
        }
        if ur:
            m["cosT"] = cosT
            m["sinT"] = sinT
        in_maps.append(m)

    res = run_bass_kernel_spmd(nc, in_maps, list(range(N_CORES)))
    out = np.empty((B, S, D_MODEL), np.float32)
    for b in range(B):
        if K2_WOPAIR:
            out[b] = (
                res.results[2 * b]["out_a"].astype(np.float32)
                + res.results[2 * b]["out_b"].astype(np.float32)
                + res.results[2 * b + 1]["out_a"].astype(np.float32)
                + res.results[2 * b + 1]["out_b"].astype(np.float32)
            )
        else:
            out[b] = res.results[2 * b]["out"] + res.results[2 * b + 1]["out"]
    return out


# revision 3
# speedup vs baseline: 1.0018x; 1.0018x over previous
"""Multi-head self-attention (B=4, S=2048, D=1024, H=16, causal + RoPE) on 8
Trainium2 NeuronCores.  v2: PE tile-pair concurrency + causal trims.

Sharding: core c = (batch b = c // 2, head-group hg = c % 2).  Host sums the
two partial outputs per batch.

Device program (fp16 matmul operands, fp32 PSUM accumulation):
  - Scores emitted as adjacent row-tiled pairs (h0 rows 0-63, h1 rows 64-127,
    different PSUM banks) -> both heads stream concurrently (~2x).
  - Wo emitted as adjacent (h, half)-alternating pairs into two PSUM banks
    po_a (out cols 0-511) / po_b (cols 512-1023): concurrent, and no
    post-merge needed (each bank is a complete output half).
  - Diagonal key blocks: scores/attn@V/mask restricted to the valid query
    range (q >= 128*j within the chunk).
  - One exp per key-block-pair over [128, 2048] PSUM -> fewer ScalarE calls.
  - ctxu evacuation carries the Z row (f32, 65 partitions); Z reshape via
    DRAM roundtrip to [64, 16], reciprocal 64 lanes wide, broadcast-read DMA.
  - proj/Wo chains woven between attention kbp iterations to fill the PE
    while ScalarE exp paces the attention pipeline.
"""

import os
import numpy as np

K2_WEAVE = os.environ.get("K2_WEAVE", "1") == "1"
K2_QSPLIT = os.environ.get("K2_QSPLIT", "1") == "1"
K2_EXP2 = os.environ.get("K2_EXP2", "1") == "1"    # single [128,2048] exp call
K2_TRIM = os.environ.get("K2_TRIM", "1") == "1"    # attnV+mask q-trim on diagonal
K2_WOPAIR = os.environ.get("K2_WOPAIR", "1") == "1"  # Wo alternating po_a/po_b

D_MODEL = 1024
NUM_HEADS = 16
D_K = 64
ROPE_THETA = 10000.0
B = 4
S = 2048
N_CORES = 8

HG_FEATS = 512          # features per core (8 heads)
FT = HG_FEATS // 128    # head pairs per core
KT = D_MODEL // 128     # contraction tiles for the projections

MM_DTYPE = "f16"
_PROGRAM_CACHE = {}


def _build_program(use_rope: bool, s: int = S):
    import concourse.tile as tile
    from concourse import bacc, mybir
    from contextlib import ExitStack

    f32 = mybir.dt.float32
    f16 = mybir.dt.float16
    mmdt = f16
    FP = mybir.ActivationFunctionType

    SC = s // 512           # 512-wide seq chunks
    ST = s // 128           # 128-wide seq tiles
    PAIRSWAP = [i ^ 1 for i in range(32)]

    nc = bacc.Bacc("TRN2", target_bir_lowering=False, debug=False)

    xT = nc.dram_tensor("xT", [D_MODEL, s], mmdt, kind="ExternalInput")
    wqT = nc.dram_tensor("wqT", [D_MODEL, HG_FEATS], mmdt, kind="ExternalInput")
    wkT = nc.dram_tensor("wkT", [D_MODEL, HG_FEATS], mmdt, kind="ExternalInput")
    wvT = nc.dram_tensor("wvT", [D_MODEL, HG_FEATS], mmdt, kind="ExternalInput")
    woT = nc.dram_tensor("woT", [HG_FEATS, D_MODEL], mmdt, kind="ExternalInput")
    if use_rope:
        cosT = nc.dram_tensor("cosT", [128, s], f16, kind="ExternalInput")
        sinT = nc.dram_tensor("sinT", [128, s], f16, kind="ExternalInput")
    if K2_WOPAIR:
        out_a = nc.dram_tensor("out_a", [s, D_MODEL], f16, kind="ExternalOutput")
        out_b = nc.dram_tensor("out_b", [s, D_MODEL], f16, kind="ExternalOutput")
    else:
        out = nc.dram_tensor("out", [s, D_MODEL], f32, kind="ExternalOutput")

    with tile.TileContext(nc) as tc, ExitStack() as ctx:
        singles = ctx.enter_context(tc.tile_pool(name="singles", bufs=1))
        stripes = ctx.enter_context(tc.tile_pool(name="stripes", bufs=2))
        tmppool = ctx.enter_context(tc.tile_pool(name="tmppool", bufs=2))
        exppool = ctx.enter_context(tc.tile_pool(name="exppool", bufs=3))
        ctxpool = ctx.enter_context(tc.tile_pool(name="ctxpool", bufs=3))
        smallp = ctx.enter_context(tc.tile_pool(name="smallp", bufs=4))
        outpool = ctx.enter_context(tc.tile_pool(name="outpool", bufs=4))
        dramp = ctx.enter_context(tc.tile_pool(name="dramp", bufs=1, space="DRAM"))
        psum = ctx.enter_context(tc.tile_pool(name="psum", bufs=1, space="PSUM"))

        # ---- persistent tiles -------------------------------------------
        wq_full = singles.tile([128, KT, HG_FEATS], mmdt, tag="wqf")
        wk_full = singles.tile([128, KT, HG_FEATS], mmdt, tag="wkf")
        wv_full = singles.tile([128, KT, HG_FEATS], mmdt, tag="wvf")
        wo_sb = singles.tile([128, FT, D_MODEL], mmdt, tag="wo")

        # Initial loads split across HWDGE queues: sync carries what the
        # first V matmuls need (wv + stripe 0); vector carries wq/wk; scalar
        # carries wo (+ rope tables).
        nc.sync.dma_start(
            out=wv_full[:, 0 : KT // 2, :],
            in_=wvT.ap()[0 : D_MODEL // 2, :].rearrange("(k p) f -> p k f", p=128),
        )
        sh_first = stripes.tile([128, KT, 512], mmdt, tag="stripe", name="sh0")
        for w in range(2):
            ks = slice(w * KT // 2, (w + 1) * KT // 2)
            nc.sync.dma_start(
                out=sh_first[:, ks, :],
                in_=xT.ap()[(ks.start * 128) : (ks.stop * 128), 0:512].rearrange(
                    "(k p) s -> p k s", p=128
                ),
            )
        nc.sync.dma_start(
            out=wv_full[:, KT // 2 : KT, :],
            in_=wvT.ap()[D_MODEL // 2 :, :].rearrange("(k p) f -> p k f", p=128),
        )
        qldma = nc.scalar if K2_QSPLIT else nc.sync
        for wsb, wdr in ((wq_full, wqT), (wk_full, wkT)):
            qldma.dma_start(
                out=wsb, in_=wdr.ap().rearrange("(k p) f -> p k f", p=128)
            )
        if use_rope:
            cos_sb = singles.tile([128, s], f16, tag="cos")
            sin_sb = singles.tile([128, s], f16, tag="sin")
            qldma.dma_start(out=cos_sb, in_=cosT.ap())
            qldma.dma_start(out=sin_sb, in_=sinT.ap())
        _wo_load = [
            lambda: qldma.dma_start(
                out=wo_sb, in_=woT.ap().rearrange("(f p) o -> p f o", p=128)
            )
        ]

        # V resident in SBUF: per (kb, hp) cols [V_h0(64) | 1 | pad | V_h1(64) | 1 | pad]
        vres = singles.tile([128, ST, FT, 132], mmdt, tag="vres")
        nc.vector.memset(
            vres[:].rearrange("p st hp (h c) -> p (st hp h) c", c=66)[:, :, 64:65],
            1.0,
        )
        # Z rows roundtrip through DRAM: [1, 1024] -> [64, 16] for a 64-lane
        # reciprocal, then 1/Z broadcast back across 64 partitions.
        ztmp = dramp.tile([FT, SC, 1, 1024], f32, tag="ztmp")
        ztmp2 = dramp.tile([FT, SC, 1, 1024], f16, tag="ztmp2")
        QTs = [
            singles.tile([128, s], mmdt, tag=f"QT{hp}", name=f"QT{hp}")
            for hp in range(FT)
        ]
        KTs = [
            singles.tile([128, s], mmdt, tag=f"KT{hp}", name=f"KT{hp}")
            for hp in range(FT)
        ]

        def load_stripe(sc):
            sh = stripes.tile([128, KT, 512], mmdt, tag="stripe", name=f"sh{sc}")
            nc.sync.dma_start(
                out=sh,
                in_=xT.ap()[:, sc * 512 : (sc + 1) * 512].rearrange(
                    "(k p) s -> p k s", p=128
                ),
            )
            return sh

        # ---- work chains (each closure emits ~1-2us of PE work) ----------
        def vproj_chain(sc, sh, j):
            def emit():
                st = sc * 4 + j
                pv = psum.tile([128, 512], f32, tag="pqk", bufs=2, name="pv")
                for kt in range(KT):
                    nc.tensor.matmul(
                        pv,
                        lhsT=sh[:, kt, j * 128 : (j + 1) * 128],
                        rhs=wv_full[:, kt, :],
                        start=(kt == 0),
                        stop=(kt == KT - 1),
                    )
                # scatter into vres: dest inner 64 f16 contiguous, 132-col
                # blocks keep h-offsets 4B-aligned (DVE evacuation)
                nc.vector.tensor_copy(
                    vres[:, st].rearrange("p hp (h c) -> p (hp h) c", c=66)[
                        :, :, 0:64
                    ],
                    pv[:].rearrange("p (x c) -> p x c", c=64),
                )
            return emit

        def qkproj_chain(sc, sh, hp, which):
            def emit():
                w_sb = wq_full if which == 0 else wk_full
                dst = QTs[hp] if which == 0 else KTs[hp]
                pq = psum.tile([128, 512], f32, tag="pqk", bufs=2, name="pq")
                for kt in range(KT):
                    nc.tensor.matmul(
                        pq,
                        lhsT=w_sb[:, kt, hp * 128 : (hp + 1) * 128],
                        rhs=sh[:, kt, :],
                        start=(kt == 0),
                        stop=(kt == KT - 1),
                    )
                dcol = dst[:, sc * 512 : (sc + 1) * 512]
                if use_rope:
                    ccol = cos_sb[:, sc * 512 : (sc + 1) * 512]
                    scol = sin_sb[:, sc * 512 : (sc + 1) * 512]
                    qsb = tmppool.tile([128, 512], f16, tag="qsb")
                    qcos = tmppool.tile([128, 512], f16, tag="qcos")
                    rot = tmppool.tile([128, 512], f16, tag="rot")
                    nc.scalar.activation(qsb, pq, FP.Copy)
                    nc.vector.stream_shuffle(rot, qsb, PAIRSWAP)
                    nc.vector.tensor_mul(qcos, qsb, ccol)
                    nc.vector.tensor_mul(rot, rot, scol)
                    nc.vector.tensor_add(dcol, qcos, rot)
                else:
                    nc.vector.tensor_copy(dcol, pq)
            return emit

        def wo_chain_base(qc, ctxn, j):
            def emit():
                st = qc * 4 + j
                osb = outpool.tile([128, 1024], f32, tag="osb")
                for half in range(2):
                    po = [
                        psum.tile([128, 512], f32, tag="pqk", bufs=2, name=f"po{h}")
                        for h in range(2)
                    ]
                    for hp in range(FT):
                        for h in range(2):
                            nc.tensor.matmul(
                                po[h],
                                lhsT=ctxn[hp][
                                    64 * h : 64 * h + 64, j * 128 : (j + 1) * 128
                                ],
                                rhs=wo_sb[
                                    64 * h : 64 * h + 64,
                                    hp,
                                    half * 512 : (half + 1) * 512,
                                ],
                                start=(hp == 0),
                                stop=(hp == FT - 1),
                                skip_group_check=True,
                            )
                    oh = osb[:, half * 512 : (half + 1) * 512]
                    nc.scalar.activation(oh, po[0], FP.Copy)
                    nc.vector.scalar_tensor_tensor(
                        out=oh,
                        in0=po[1],
                        scalar=1.0,
                        in1=oh,
                        op0=mybir.AluOpType.mult,
                        op1=mybir.AluOpType.add,
                    )
                nc.sync.dma_start(
                    out=out.ap()[st * 128 : (st + 1) * 128, :], in_=osb
                )
            return emit

        def wo_chain(qc, ctxn, j, epilogue=False):
            if not K2_WOPAIR:
                return wo_chain_base(qc, ctxn, j)

            def emit():
                st = qc * 4 + j
                # 4 chains, each with a consistent PE row group:
                #   h0 chains (rows 0-63)  -> osb_a -> out_a
                #   h1 chains (rows 64-127) -> osb_b -> out_b
                # (h0, half) and (h1, half) emitted adjacently -> concurrent.
                osb_a = outpool.tile([128, 1024], f16, tag="osba")
                osb_b = outpool.tile([128, 1024], f16, tag="osbb")
                for half in range(2):
                    if epilogue and (2 * j + half) % 2 == 1:
                        pop = psum.tile(
                            [128, 1024], f32, tag="oacc", bufs=1, name="pop"
                        )
                        po0 = pop[:, 0:512]
                        po1 = pop[:, 512:1024]
                    else:
                        po0 = psum.tile([128, 512], f32, tag="pqk", bufs=2, name="po0")
                        po1 = psum.tile([128, 512], f32, tag="pqk", bufs=2, name="po1")
                    for hp in range(FT):
                        for h, po in ((0, po0), (1, po1)):
                            nc.tensor.matmul(
                                po,
                                lhsT=ctxn[hp][
                                    64 * h : 64 * h + 64, j * 128 : (j + 1) * 128
                                ],
                                rhs=wo_sb[
                                    64 * h : 64 * h + 64,
                                    hp,
                                    half * 512 : (half + 1) * 512,
                                ],
                                start=(hp == 0),
                                stop=(hp == FT - 1),
                                skip_group_check=True,
                            )
                    # evacuate: one on ScalarE, one on DVE (engine balance)
                    nc.scalar.activation(
                        osb_a[:, half * 512 : (half + 1) * 512], po0, FP.Copy
                    )
                    nc.vector.tensor_copy(
                        osb_b[:, half * 512 : (half + 1) * 512], po1
                    )
                nc.sync.dma_start(
                    out=out_a.ap()[st * 128 : (st + 1) * 128, :], in_=osb_a
                )
                nc.sync.dma_start(
                    out=out_b.ap()[st * 128 : (st + 1) * 128, :], in_=osb_b
                )
            return emit

        # ---- attention ---------------------------------------------------
        def do_attn(qc, work):
            nkb = 4 * qc + 4
            ctxn = [
                ctxpool.tile([128, 512], f16, tag=f"ctxn{hp}", name=f"ctxn{hp}")
                for hp in range(FT)
            ]
            for hp in range(FT):
                QT = QTs[hp]
                KTt = KTs[hp]
                oacc = psum.tile([128, 1024], f32, tag="oacc", bufs=1, name="oacc")
                def emit_attnv(kb, expt):
                    j = kb - 4 * qc
                    q0 = 128 * j if (j >= 0 and K2_TRIM) else 0
                    for h in range(2):
                        nc.tensor.matmul(
                            oacc[0:65, 512 * h + q0 : 512 * h + 512],
                            lhsT=vres[:, kb, hp, 66 * h : 66 * h + 65],
                            rhs=expt[:, 512 * h + q0 : 512 * h + 512],
                            start=(kb == 0),
                            stop=(kb == nkb - 1),
                            skip_group_check=True,
                        )

                prev = None
                for kb in range(nkb):
                    if kb % 2 == 0:
                        _drain_due(work, (qc, hp, kb // 2))
                    # scores [128, 1024]: [h0 | h1], double-buffered
                    scr = psum.tile(
                        [128, 1024], f32, tag="scores", bufs=2, name="scores"
                    )
                    for h in range(2):
                        nc.tensor.matmul(
                            scr[:, 512 * h : 512 * h + 512],
                            lhsT=KTt[
                                64 * h : 64 * h + 64, kb * 128 : (kb + 1) * 128
                            ],
                            rhs=QT[
                                64 * h : 64 * h + 64,
                                qc * 512 : (qc + 1) * 512,
                            ],
                            start=True,
                            stop=True,
                            skip_group_check=True,
                        )
                    expt = exppool.tile([128, 1024], mmdt, tag="expt")
                    nc.scalar.activation(expt, scr, FP.Exp, scale=0.125)
                    j = kb - 4 * qc
                    if j >= 0:  # diagonal tile: causal mask post-exp
                        q0 = 128 * j if K2_TRIM else 0
                        for h in range(2):
                            sl = expt[:, 512 * h + q0 : 512 * h + 512]
                            nc.gpsimd.affine_select(
                                out=sl,
                                in_=sl,
                                compare_op=mybir.AluOpType.is_ge,
                                fill=0.0,
                                base=-(128 * j - q0),
                                pattern=[[1, 512 - q0]],
                                channel_multiplier=-1,
                            )
                    # previous kb's attn@V (exp/mask latency covered)
                    if prev is not None:
                        emit_attnv(*prev)
                    prev = (kb, expt)
                emit_attnv(*prev)
                # ---- evacuate ctx+Z, normalize -------------------------
                ctxu = smallp.tile([128, 1024], f32, tag="ctxu")
                nc.vector.tensor_copy(ctxu[0:65, :], oacc[0:65, :])
                dmae = nc.sync
                dmae.dma_start(out=ztmp[hp, qc], in_=ctxu[64:65, :])
                zT = smallp.tile([64, 16], f32, tag="zT")
                dmae.dma_start(
                    out=zT, in_=ztmp[hp, qc, 0].rearrange("(p g) -> p g", p=64)
                )
                zinvT = smallp.tile([64, 16], f32, tag="zinvT")
                nc.vector.reciprocal(zinvT, zT)
                zinvT16 = smallp.tile([64, 16], f16, tag="zinvT16")
                nc.vector.tensor_copy(zinvT16, zinvT)
                dmae.dma_start(
                    out=ztmp2[hp, qc, 0].rearrange("(p g) -> p g", p=64),
                    in_=zinvT16,
                )
                zbc_sb = smallp.tile([128, 1024], f16, tag="zbcsb")
                dmae.dma_start(
                    out=zbc_sb[0:64, :],
                    in_=ztmp2[hp, qc].broadcast_to([64, 1024]),
                )
                for h in range(2):
                    nc.vector.scalar_tensor_tensor(
                        out=ctxn[hp][64 * h : 64 * h + 64, :],
                        in0=ctxu[0:64, 512 * h : 512 * h + 512],
                        scalar=1.0,
                        in1=zbc_sb[0:64, 512 * h : 512 * h + 512],
                        op0=mybir.AluOpType.mult,
                        op1=mybir.AluOpType.mult,
                    )
            return ctxn

        # ---- deadline-scheduled emission ---------------------------------
        # Every chain gets an emission deadline (qc, hp, kbp): it is emitted
        # just before that attention step.  Deadlines place each chain where
        # the PE needs fill (late chunks have more exp time to cover), always
        # at-or-before the step that first reads the chain's outputs.
        work = []

        def _drain_due(work, step):
            i = 0
            while i < len(work):
                dl, em = work[i]
                if dl <= step:
                    em()
                    work.pop(i)
                else:
                    i += 1

        # proj(0): V j0/j1 + QK hp0 land in the prologue (deadline (0,0,0));
        # the rest weave into attn(0) at their first-read step.
        PROJ_DL = {
            0: {("v", 0): (0, 0, 0), ("v", 1): (0, 0, 0),
                ("v", 2): (0, 0, 1), ("v", 3): (0, 0, 1),
                ("qk", 0, 0): (0, 0, 0), ("qk", 0, 1): (0, 0, 0),
                ("qk", 1, 0): (0, 1, 0), ("qk", 1, 1): (0, 1, 0),
                ("qk", 2, 0): (0, 2, 0), ("qk", 2, 1): (0, 2, 0),
                ("qk", 3, 0): (0, 3, 0), ("qk", 3, 1): (0, 3, 0)},
            # proj(1): V + QK(hp0) inside attn(0); the rest early in attn(1)
            1: {("v", 0): (0, 0, 1), ("v", 1): (0, 1, 1),
                ("v", 2): (0, 2, 1), ("v", 3): (0, 3, 1),
                ("qk", 0, 0): (0, 1, 1), ("qk", 0, 1): (0, 2, 1),
                ("qk", 1, 0): (1, 0, 0), ("qk", 1, 1): (1, 0, 1),
                ("qk", 2, 0): (1, 1, 0), ("qk", 2, 1): (1, 1, 1),
                ("qk", 3, 0): (1, 2, 0), ("qk", 3, 1): (1, 2, 1)},
            # proj(2): spread across attn(1)
            2: {("v", 0): (1, 0, 1), ("v", 1): (1, 1, 1),
                ("v", 2): (1, 2, 1), ("v", 3): (1, 3, 1),
                ("qk", 0, 0): (1, 0, 2), ("qk", 0, 1): (1, 0, 3),
                ("qk", 1, 0): (1, 1, 2), ("qk", 1, 1): (1, 1, 3),
                ("qk", 2, 0): (1, 2, 2), ("qk", 2, 1): (1, 2, 3),
                ("qk", 3, 0): (1, 3, 2), ("qk", 3, 1): (1, 3, 3)},
            # proj(3): V + QK(hp0) spread across attn(2); QK(hp1-3) spill
            # into attn(3) just before each hp needs them
            3: {("v", 0): (2, 0, 2), ("v", 1): (2, 0, 5),
                ("v", 2): (2, 1, 1), ("v", 3): (2, 1, 4),
                ("qk", 0, 0): (2, 0, 0), ("qk", 0, 1): (2, 2, 0),
                ("qk", 1, 0): (3, 0, 1), ("qk", 1, 1): (3, 0, 4),
                ("qk", 2, 0): (3, 1, 1), ("qk", 2, 1): (3, 1, 5),
                ("qk", 3, 0): (3, 2, 2), ("qk", 3, 1): (3, 2, 4)},
        }
        # wo(sc) spread: wo(0) into attn(2); wo(1)/wo(2) into attn(3)
        WO_DL = {
            0: [(2, 2, 2), (2, 2, 5), (2, 3, 1), (2, 3, 3)],
            1: [(3, 0, 6), (3, 1, 3), (3, 1, 7), (3, 2, 6)],
            2: [(3, 3, 1), (3, 3, 3), (3, 3, 5), (3, 3, 7)],
        }

        def add_proj_work(sc, sh):
            dls = PROJ_DL[sc]
            for j in range(4):
                work.append((dls[("v", j)], vproj_chain(sc, sh, j)))
            for hp in range(FT):
                for w in (0, 1):
                    work.append((dls[("qk", hp, w)], qkproj_chain(sc, sh, hp, w)))

        add_proj_work(0, sh_first)
        work.append(((0, 2, 0), _wo_load[0]))
        ctxs = {}
        for sc in range(SC):
            if sc + 1 < SC:
                sh = load_stripe(sc + 1)
                add_proj_work(sc + 1, sh)
            if sc - 1 >= 0 and sc - 1 in WO_DL:
                prev_ctx = ctxs[sc - 1]
                for j in range(4):
                    work.append((WO_DL[sc - 1][j], wo_chain(sc - 1, prev_ctx, j)))
            ctxs[sc] = do_attn(sc, work)
        for dl, em in work:  # anything left (defensive)
            em()
        work.clear()
        last_ctx = ctxs[SC - 1]
        for em in [wo_chain(SC - 1, last_ctx, j, epilogue=True) for j in range(4)]:
            em()

    nc.compile()
    return nc


def _round_f32r(a):
    """Round fp32 array to the PE's FP32R format (RNE at 12 low mantissa bits)."""
    u = np.ascontiguousarray(a, np.float32).view(np.uint32).astype(np.uint64)
    low = u & 0xFFF
    up = (low > 0x800) | ((low == 0x800) & (((u >> 12) & 1) == 1))
    r = (u & ~np.uint64(0xFFF)) + np.where(up, 0x1000, 0)
    return r.astype(np.uint32).view(np.float32)


def _to_f16(a):
    return np.ascontiguousarray(a, np.float16)


def _rope_tables(s: int):
    inv_freq = 1.0 / (ROPE_THETA ** (np.arange(0, D_K, 2, dtype=np.float64) / D_K))
    angles = np.arange(s, dtype=np.float64)[:, None] * inv_freq[None, :]  # [s, 32]
    cos = np.cos(angles).astype(np.float32)  # [s, 32]
    sin = np.sin(angles).astype(np.float32)
    cosT = np.empty((D_K, s), np.float32)
    sinT = np.empty((D_K, s), np.float32)
    cosT[0::2] = cos.T
    cosT[1::2] = cos.T
    sinT[0::2] = -sin.T
    sinT[1::2] = sin.T
    return (
        np.ascontiguousarray(np.vstack([cosT, cosT])).astype(np.float16),
        np.ascontiguousarray(np.vstack([sinT, sinT])).astype(np.float16),
    )


def kernel(x, Wq, Wk, Wv, Wo, use_rope):
    from concourse.bass_utils import run_bass_kernel_spmd

    x = np.asarray(x, dtype=np.float32)
    ur = bool(int(np.asarray(use_rope)))
    key = (ur, S)
    if key not in _PROGRAM_CACHE:
        _PROGRAM_CACHE[key] = _build_program(ur, S)
    nc = _PROGRAM_CACHE[key]

    if ur:
        cosT, sinT = _rope_tables(S)

    in_maps = []
    for c in range(N_CORES):
        b, hg = c // 2, c % 2
        sl = slice(hg * HG_FEATS, (hg + 1) * HG_FEATS)
        cv = _round_f32r if MM_DTYPE == "f32r" else _to_f16
        m = {
            "xT": cv(x[b].T),
            "wqT": cv(np.asarray(Wq, np.float32)[sl, :].T),
            "wkT": cv(np.asarray(Wk, np.float32)[sl, :].T),
            "wvT": cv(np.asarray(Wv, np.float32)[sl, :].T),
            "woT": cv(np.asarray(Wo, np.float32)[:, sl].T),
        }
        if ur:
            m["cosT"] = cosT
            m["sinT"] = sinT
        in_maps.append(m)

    res = run_bass_kernel_spmd(nc, in_maps, list(range(N_CORES)))
    out = np.empty((B, S, D_MODEL), np.float32)
    for b in range(B):
        if K2_WOPAIR:
            out[b] = (
                res.results[2 * b]["out_a"].astype(np.float32)
                + res.results[2 * b]["out_b"].astype(np.float32)
                + res.results[2 * b + 1]["out_a"].astype(np.float32)
                + res.results[2 * b + 1]["out_b"].astype(np.float32)
            )
        else:
            out[b] = res.results[2 * b]["out"] + res.results[2 * b + 1]["out"]
    return out


# revision 4
# speedup vs baseline: 1.0100x; 1.0082x over previous
"""Multi-head self-attention (B=4, S=2048, D=1024, H=16, causal + RoPE) on 8
Trainium2 NeuronCores.  v2: PE tile-pair concurrency + causal trims.

Sharding: core c = (batch b = c // 2, head-group hg = c % 2).  Host sums the
two partial outputs per batch.

Device program (fp16 matmul operands, fp32 PSUM accumulation):
  - Scores emitted as adjacent row-tiled pairs (h0 rows 0-63, h1 rows 64-127,
    different PSUM banks) -> both heads stream concurrently (~2x).
  - Wo emitted as adjacent (h, half)-alternating pairs into two PSUM banks
    po_a (out cols 0-511) / po_b (cols 512-1023): concurrent, and no
    post-merge needed (each bank is a complete output half).
  - Diagonal key blocks: scores/attn@V/mask restricted to the valid query
    range (q >= 128*j within the chunk).
  - One exp per key-block-pair over [128, 2048] PSUM -> fewer ScalarE calls.
  - ctxu evacuation carries the Z row (f32, 65 partitions); Z reshape via
    DRAM roundtrip to [64, 16], reciprocal 64 lanes wide, broadcast-read DMA.
  - proj/Wo chains woven between attention kbp iterations to fill the PE
    while ScalarE exp paces the attention pipeline.
"""

import os
import numpy as np

K2_WEAVE = os.environ.get("K2_WEAVE", "1") == "1"
K2_QSPLIT = os.environ.get("K2_QSPLIT", "1") == "1"
K2_EXP2 = os.environ.get("K2_EXP2", "1") == "1"    # single [128,2048] exp call
K2_TRIM = os.environ.get("K2_TRIM", "1") == "1"    # attnV+mask q-trim on diagonal
K2_WOPAIR = os.environ.get("K2_WOPAIR", "1") == "1"  # Wo alternating po_a/po_b

D_MODEL = 1024
NUM_HEADS = 16
D_K = 64
ROPE_THETA = 10000.0
B = 4
S = 2048
N_CORES = 8

HG_FEATS = 512          # features per core (8 heads)
FT = HG_FEATS // 128    # head pairs per core
KT = D_MODEL // 128     # contraction tiles for the projections

MM_DTYPE = "f16"
_PROGRAM_CACHE = {}


def _build_program(use_rope: bool, s: int = S):
    import concourse.tile as tile
    from concourse import bacc, mybir
    from contextlib import ExitStack

    f32 = mybir.dt.float32
    f16 = mybir.dt.float16
    mmdt = f16
    FP = mybir.ActivationFunctionType

    SC = s // 512           # 512-wide seq chunks
    ST = s // 128           # 128-wide seq tiles
    PAIRSWAP = [i ^ 1 for i in range(32)]

    nc = bacc.Bacc("TRN2", target_bir_lowering=False, debug=False)

    xT = nc.dram_tensor("xT", [D_MODEL, s], mmdt, kind="ExternalInput")
    wqT = nc.dram_tensor("wqT", [D_MODEL, HG_FEATS], mmdt, kind="ExternalInput")
    wkT = nc.dram_tensor("wkT", [D_MODEL, HG_FEATS], mmdt, kind="ExternalInput")
    wvT = nc.dram_tensor("wvT", [D_MODEL, HG_FEATS], mmdt, kind="ExternalInput")
    woT = nc.dram_tensor("woT", [HG_FEATS, D_MODEL], mmdt, kind="ExternalInput")
    if use_rope:
        cosT = nc.dram_tensor("cosT", [128, s], f16, kind="ExternalInput")
        sinT = nc.dram_tensor("sinT", [128, s], f16, kind="ExternalInput")
    if K2_WOPAIR:
        out_a = nc.dram_tensor("out_a", [s, D_MODEL], f16, kind="ExternalOutput")
        out_b = nc.dram_tensor("out_b", [s, D_MODEL], f16, kind="ExternalOutput")
    else:
        out = nc.dram_tensor("out", [s, D_MODEL], f32, kind="ExternalOutput")

    with tile.TileContext(nc) as tc, ExitStack() as ctx:
        singles = ctx.enter_context(tc.tile_pool(name="singles", bufs=1))
        stripes = ctx.enter_context(tc.tile_pool(name="stripes", bufs=2))
        tmppool = ctx.enter_context(tc.tile_pool(name="tmppool", bufs=2))
        exppool = ctx.enter_context(tc.tile_pool(name="exppool", bufs=3))
        ctxpool = ctx.enter_context(tc.tile_pool(name="ctxpool", bufs=3))
        smallp = ctx.enter_context(tc.tile_pool(name="smallp", bufs=4))
        outpool = ctx.enter_context(tc.tile_pool(name="outpool", bufs=4))
        dramp = ctx.enter_context(tc.tile_pool(name="dramp", bufs=1, space="DRAM"))
        psum = ctx.enter_context(tc.tile_pool(name="psum", bufs=1, space="PSUM"))

        # ---- persistent tiles -------------------------------------------
        wq_full = singles.tile([128, KT, HG_FEATS], mmdt, tag="wqf")
        wk_full = singles.tile([128, KT, HG_FEATS], mmdt, tag="wkf")
        wv_full = singles.tile([128, KT, HG_FEATS], mmdt, tag="wvf")
        wo_sb = singles.tile([128, FT, D_MODEL], mmdt, tag="wo")

        # Initial loads split across HWDGE queues: sync carries what the
        # first V matmuls need (wv + stripe 0); vector carries wq/wk; scalar
        # carries wo (+ rope tables).
        nc.sync.dma_start(
            out=wv_full[:, 0 : KT // 2, :],
            in_=wvT.ap()[0 : D_MODEL // 2, :].rearrange("(k p) f -> p k f", p=128),
        )
        sh_first = stripes.tile([128, KT, 512], mmdt, tag="stripe", name="sh0")
        for w in range(2):
            ks = slice(w * KT // 2, (w + 1) * KT // 2)
            nc.sync.dma_start(
                out=sh_first[:, ks, :],
                in_=xT.ap()[(ks.start * 128) : (ks.stop * 128), 0:512].rearrange(
                    "(k p) s -> p k s", p=128
                ),
            )
        nc.sync.dma_start(
            out=wv_full[:, KT // 2 : KT, :],
            in_=wvT.ap()[D_MODEL // 2 :, :].rearrange("(k p) f -> p k f", p=128),
        )
        qldma = nc.scalar if K2_QSPLIT else nc.sync
        for wsb, wdr in ((wq_full, wqT), (wk_full, wkT)):
            qldma.dma_start(
                out=wsb, in_=wdr.ap().rearrange("(k p) f -> p k f", p=128)
            )
        if use_rope:
            cos_sb = singles.tile([128, s], f16, tag="cos")
            sin_sb = singles.tile([128, s], f16, tag="sin")
            qldma.dma_start(out=cos_sb, in_=cosT.ap())
            qldma.dma_start(out=sin_sb, in_=sinT.ap())
        _wo_load = [
            lambda: qldma.dma_start(
                out=wo_sb, in_=woT.ap().rearrange("(f p) o -> p f o", p=128)
            )
        ]

        # V resident in SBUF: per (kb, hp) cols [V_h0(64) | 1 | pad | V_h1(64) | 1 | pad]
        vres = singles.tile([128, ST, FT, 132], mmdt, tag="vres")
        nc.vector.memset(
            vres[:].rearrange("p st hp (h c) -> p (st hp h) c", c=66)[:, :, 64:65],
            1.0,
        )
        # Z rows roundtrip through DRAM: [1, 1024] -> [64, 16] for a 64-lane
        # reciprocal, then 1/Z broadcast back across 64 partitions.
        ztmp = dramp.tile([FT, SC, 1, 1024], f32, tag="ztmp")
        ztmp2 = dramp.tile([FT, SC, 1, 1024], f16, tag="ztmp2")
        QTs = [
            singles.tile([128, s], mmdt, tag=f"QT{hp}", name=f"QT{hp}")
            for hp in range(FT)
        ]
        KTs = [
            singles.tile([128, s], mmdt, tag=f"KT{hp}", name=f"KT{hp}")
            for hp in range(FT)
        ]

        def load_stripe(sc):
            sh = stripes.tile([128, KT, 512], mmdt, tag="stripe", name=f"sh{sc}")
            nc.sync.dma_start(
                out=sh,
                in_=xT.ap()[:, sc * 512 : (sc + 1) * 512].rearrange(
                    "(k p) s -> p k s", p=128
                ),
            )
            return sh

        # ---- work chains (each closure emits ~1-2us of PE work) ----------
        def vproj_chain(sc, sh, j):
            def emit():
                st = sc * 4 + j
                pv = psum.tile([128, 512], f32, tag="pqk", bufs=2, name="pv")
                for kt in range(KT):
                    nc.tensor.matmul(
                        pv,
                        lhsT=sh[:, kt, j * 128 : (j + 1) * 128],
                        rhs=wv_full[:, kt, :],
                        start=(kt == 0),
                        stop=(kt == KT - 1),
                    )
                # scatter into vres: dest inner 64 f16 contiguous, 132-col
                # blocks keep h-offsets 4B-aligned (DVE evacuation)
                nc.vector.tensor_copy(
                    vres[:, st].rearrange("p hp (h c) -> p (hp h) c", c=66)[
                        :, :, 0:64
                    ],
                    pv[:].rearrange("p (x c) -> p x c", c=64),
                )
            return emit

        def qkproj_chain(sc, sh, hp, which):
            def emit():
                w_sb = wq_full if which == 0 else wk_full
                dst = QTs[hp] if which == 0 else KTs[hp]
                pq = psum.tile([128, 512], f32, tag="pqk", bufs=2, name="pq")
                for kt in range(KT):
                    nc.tensor.matmul(
                        pq,
                        lhsT=w_sb[:, kt, hp * 128 : (hp + 1) * 128],
                        rhs=sh[:, kt, :],
                        start=(kt == 0),
                        stop=(kt == KT - 1),
                    )
                dcol = dst[:, sc * 512 : (sc + 1) * 512]
                if use_rope:
                    ccol = cos_sb[:, sc * 512 : (sc + 1) * 512]
                    scol = sin_sb[:, sc * 512 : (sc + 1) * 512]
                    qsb = tmppool.tile([128, 512], f16, tag="qsb")
                    qcos = tmppool.tile([128, 512], f16, tag="qcos")
                    rot = tmppool.tile([128, 512], f16, tag="rot")
                    nc.scalar.activation(qsb, pq, FP.Copy)
                    nc.vector.stream_shuffle(rot, qsb, PAIRSWAP)
                    nc.vector.tensor_mul(qcos, qsb, ccol)
                    nc.vector.tensor_mul(rot, rot, scol)
                    nc.vector.tensor_add(dcol, qcos, rot)
                else:
                    nc.vector.tensor_copy(dcol, pq)
            return emit

        def wo_chain_base(qc, ctxn, j):
            def emit():
                st = qc * 4 + j
                osb = outpool.tile([128, 1024], f32, tag="osb")
                for half in range(2):
                    po = [
                        psum.tile([128, 512], f32, tag="pqk", bufs=2, name=f"po{h}")
                        for h in range(2)
                    ]
                    for hp in range(FT):
                        for h in range(2):
                            nc.tensor.matmul(
                                po[h],
                                lhsT=ctxn[hp][
                                    64 * h : 64 * h + 64, j * 128 : (j + 1) * 128
                                ],
                                rhs=wo_sb[
                                    64 * h : 64 * h + 64,
                                    hp,
                                    half * 512 : (half + 1) * 512,
                                ],
                                start=(hp == 0),
                                stop=(hp == FT - 1),
                                skip_group_check=True,
                            )
                    oh = osb[:, half * 512 : (half + 1) * 512]
                    nc.scalar.activation(oh, po[0], FP.Copy)
                    nc.vector.scalar_tensor_tensor(
                        out=oh,
                        in0=po[1],
                        scalar=1.0,
                        in1=oh,
                        op0=mybir.AluOpType.mult,
                        op1=mybir.AluOpType.add,
                    )
                nc.sync.dma_start(
                    out=out.ap()[st * 128 : (st + 1) * 128, :], in_=osb
                )
            return emit

        def wo_chain(qc, ctxn, j, epilogue=False):
            if not K2_WOPAIR:
                return wo_chain_base(qc, ctxn, j)

            def emit():
                st = qc * 4 + j
                # 4 chains, each with a consistent PE row group:
                #   h0 chains (rows 0-63)  -> osb_a -> out_a
                #   h1 chains (rows 64-127) -> osb_b -> out_b
                # (h0, half) and (h1, half) emitted adjacently -> concurrent.
                osb_a = outpool.tile([128, 1024], f16, tag="osba")
                osb_b = outpool.tile([128, 1024], f16, tag="osbb")
                for half in range(2):
                    if epilogue and (2 * j + half) % 2 == 1:
                        pop = psum.tile(
                            [128, 1024], f32, tag="oacc", bufs=1, name="pop"
                        )
                        po0 = pop[:, 0:512]
                        po1 = pop[:, 512:1024]
                    else:
                        po0 = psum.tile([128, 512], f32, tag="pqk", bufs=2, name="po0")
                        po1 = psum.tile([128, 512], f32, tag="pqk", bufs=2, name="po1")
                    for hp in range(FT):
                        for h, po in ((0, po0), (1, po1)):
                            nc.tensor.matmul(
                                po,
                                lhsT=ctxn[hp][
                                    64 * h : 64 * h + 64, j * 128 : (j + 1) * 128
                                ],
                                rhs=wo_sb[
                                    64 * h : 64 * h + 64,
                                    hp,
                                    half * 512 : (half + 1) * 512,
                                ],
                                start=(hp == 0),
                                stop=(hp == FT - 1),
                                skip_group_check=True,
                            )
                    # evacuate: one on ScalarE, one on DVE (engine balance)
                    nc.scalar.activation(
                        osb_a[:, half * 512 : (half + 1) * 512], po0, FP.Copy
                    )
                    nc.vector.tensor_copy(
                        osb_b[:, half * 512 : (half + 1) * 512], po1
                    )
                nc.sync.dma_start(
                    out=out_a.ap()[st * 128 : (st + 1) * 128, :], in_=osb_a
                )
                nc.sync.dma_start(
                    out=out_b.ap()[st * 128 : (st + 1) * 128, :], in_=osb_b
                )
            return emit

        # ---- attention ---------------------------------------------------
        def do_attn(qc, work):
            nkb = 4 * qc + 4
            ctxn = [
                ctxpool.tile([128, 512], f16, tag=f"ctxn{hp}", name=f"ctxn{hp}")
                for hp in range(FT)
            ]
            for hp in range(FT):
                QT = QTs[hp]
                KTt = KTs[hp]
                oacc = psum.tile([128, 1024], f32, tag="oacc", bufs=1, name="oacc")
                def emit_attnv(kb, expt):
                    j = kb - 4 * qc
                    q0 = 128 * j if (j >= 0 and K2_TRIM) else 0
                    for h in range(2):
                        nc.tensor.matmul(
                            oacc[0:65, 512 * h + q0 : 512 * h + 512],
                            lhsT=vres[:, kb, hp, 66 * h : 66 * h + 65],
                            rhs=expt[:, 512 * h + q0 : 512 * h + 512],
                            start=(kb == 0),
                            stop=(kb == nkb - 1),
                            skip_group_check=True,
                        )

                prev = None
                for kb in range(nkb):
                    if kb % 2 == 0:
                        _drain_due(work, (qc, hp, kb // 2))
                    # scores [128, 1024]: [h0 | h1], double-buffered
                    scr = psum.tile(
                        [128, 1024], f32, tag="scores", bufs=2, name="scores"
                    )
                    for h in range(2):
                        nc.tensor.matmul(
                            scr[:, 512 * h : 512 * h + 512],
                            lhsT=KTt[
                                64 * h : 64 * h + 64, kb * 128 : (kb + 1) * 128
                            ],
                            rhs=QT[
                                64 * h : 64 * h + 64,
                                qc * 512 : (qc + 1) * 512,
                            ],
                            start=True,
                            stop=True,
                            skip_group_check=True,
                        )
                    expt = exppool.tile([128, 1024], mmdt, tag="expt")
                    nc.scalar.activation(expt, scr, FP.Exp, scale=0.125)
                    j = kb - 4 * qc
                    if j >= 0:  # diagonal tile: causal mask post-exp
                        q0 = 128 * j if K2_TRIM else 0
                        for h in range(2):
                            sl = expt[:, 512 * h + q0 : 512 * h + 512]
                            nc.gpsimd.affine_select(
                                out=sl,
                                in_=sl,
                                compare_op=mybir.AluOpType.is_ge,
                                fill=0.0,
                                base=-(128 * j - q0),
                                pattern=[[1, 512 - q0]],
                                channel_multiplier=-1,
                            )
                    # previous kb's attn@V (exp/mask latency covered)
                    if prev is not None:
                        emit_attnv(*prev)
                    prev = (kb, expt)
                emit_attnv(*prev)
                # ---- evacuate ctx+Z, normalize -------------------------
                ctxu = smallp.tile([128, 1024], f32, tag="ctxu")
                nc.vector.tensor_copy(ctxu[0:65, :], oacc[0:65, :])
                dmae = nc.sync
                dmae.dma_start(out=ztmp[hp, qc], in_=ctxu[64:65, :])
                zT = smallp.tile([64, 16], f32, tag="zT")
                dmae.dma_start(
                    out=zT, in_=ztmp[hp, qc, 0].rearrange("(p g) -> p g", p=64)
                )
                zinvT = smallp.tile([64, 16], f32, tag="zinvT")
                nc.vector.reciprocal(zinvT, zT)
                zinvT16 = smallp.tile([64, 16], f16, tag="zinvT16")
                nc.vector.tensor_copy(zinvT16, zinvT)
                dmae.dma_start(
                    out=ztmp2[hp, qc, 0].rearrange("(p g) -> p g", p=64),
                    in_=zinvT16,
                )
                zbc_sb = smallp.tile([128, 1024], f16, tag="zbcsb")
                dmae.dma_start(
                    out=zbc_sb[0:64, :],
                    in_=ztmp2[hp, qc].broadcast_to([64, 1024]),
                )
                for h in range(2):
                    nc.vector.scalar_tensor_tensor(
                        out=ctxn[hp][64 * h : 64 * h + 64, :],
                        in0=ctxu[0:64, 512 * h : 512 * h + 512],
                        scalar=1.0,
                        in1=zbc_sb[0:64, 512 * h : 512 * h + 512],
                        op0=mybir.AluOpType.mult,
                        op1=mybir.AluOpType.mult,
                    )
            return ctxn

        # ---- deadline-scheduled emission ---------------------------------
        # Every chain gets an emission deadline (qc, hp, kbp): it is emitted
        # just before that attention step.  Deadlines place each chain where
        # the PE needs fill (late chunks have more exp time to cover), always
        # at-or-before the step that first reads the chain's outputs.
        work = []

        def _drain_due(work, step):
            i = 0
            while i < len(work):
                dl, em = work[i]
                if dl <= step:
                    em()
                    work.pop(i)
                else:
                    i += 1

        # proj(0): V j0/j1 + QK hp0 land in the prologue (deadline (0,0,0));
        # the rest weave into attn(0) at their first-read step.
        PROJ_DL = {
            0: {("v", 0): (0, 0, 0), ("v", 1): (0, 0, 0),
                ("v", 2): (0, 0, 1), ("v", 3): (0, 0, 1),
                ("qk", 0, 0): (0, 0, 0), ("qk", 0, 1): (0, 0, 0),
                ("qk", 1, 0): (0, 1, 0), ("qk", 1, 1): (0, 1, 0),
                ("qk", 2, 0): (0, 2, 0), ("qk", 2, 1): (0, 2, 0),
                ("qk", 3, 0): (0, 3, 0), ("qk", 3, 1): (0, 3, 0)},
            # proj(1): V + QK(hp0) inside attn(0); the rest early in attn(1)
            1: {("v", 0): (0, 0, 1), ("v", 1): (0, 1, 1),
                ("v", 2): (0, 2, 1), ("v", 3): (0, 3, 1),
                ("qk", 0, 0): (0, 1, 1), ("qk", 0, 1): (0, 2, 1),
                ("qk", 1, 0): (1, 0, 0), ("qk", 1, 1): (1, 0, 1),
                ("qk", 2, 0): (1, 1, 0), ("qk", 2, 1): (1, 1, 1),
                ("qk", 3, 0): (1, 2, 0), ("qk", 3, 1): (1, 2, 1)},
            # proj(2): spread across attn(1)
            2: {("v", 0): (1, 0, 1), ("v", 1): (1, 1, 1),
                ("v", 2): (1, 2, 1), ("v", 3): (1, 3, 1),
                ("qk", 0, 0): (1, 0, 2), ("qk", 0, 1): (1, 0, 3),
                ("qk", 1, 0): (1, 1, 2), ("qk", 1, 1): (1, 1, 3),
                ("qk", 2, 0): (1, 2, 2), ("qk", 2, 1): (1, 2, 3),
                ("qk", 3, 0): (1, 3, 2), ("qk", 3, 1): (1, 3, 3)},
            # proj(3): V + QK(hp0) spread across attn(2); QK(hp1-3) spill
            # into attn(3) just before each hp needs them
            3: {("v", 0): (2, 0, 2), ("v", 1): (2, 0, 5),
                ("v", 2): (2, 1, 1), ("v", 3): (2, 1, 4),
                ("qk", 0, 0): (2, 0, 0), ("qk", 0, 1): (2, 2, 0),
                ("qk", 1, 0): (3, 0, 1), ("qk", 1, 1): (3, 0, 4),
                ("qk", 2, 0): (3, 1, 1), ("qk", 2, 1): (3, 1, 5),
                ("qk", 3, 0): (3, 2, 2), ("qk", 3, 1): (3, 2, 4)},
        }
        # wo(sc) spread: wo(0) into attn(2); wo(1)/wo(2) into attn(3)
        WO_DL = {
            0: [(2, 2, 2), (2, 2, 5), (2, 3, 1), (2, 3, 3)],
            1: [(3, 0, 6), (3, 1, 3), (3, 1, 7), (3, 2, 6)],
            # wo(2): no in-loop deadline -> emitted at the post-loop flush,
            # leaving ready PE work to cover the last Z-chain latency
            2: [(3, 99, 0), (3, 99, 0), (3, 99, 0), (3, 99, 0)],
        }

        def add_proj_work(sc, sh):
            dls = PROJ_DL[sc]
            for j in range(4):
                work.append((dls[("v", j)], vproj_chain(sc, sh, j)))
            for hp in range(FT):
                for w in (0, 1):
                    work.append((dls[("qk", hp, w)], qkproj_chain(sc, sh, hp, w)))

        add_proj_work(0, sh_first)
        work.append(((0, 2, 0), _wo_load[0]))
        ctxs = {}
        for sc in range(SC):
            if sc + 1 < SC:
                sh = load_stripe(sc + 1)
                add_proj_work(sc + 1, sh)
            if sc - 1 >= 0 and sc - 1 in WO_DL:
                prev_ctx = ctxs[sc - 1]
                for j in range(4):
                    work.append((WO_DL[sc - 1][j], wo_chain(sc - 1, prev_ctx, j)))
            ctxs[sc] = do_attn(sc, work)
        for dl, em in work:  # anything left (defensive)
            em()
        work.clear()
        last_ctx = ctxs[SC - 1]
        for em in [wo_chain(SC - 1, last_ctx, j, epilogue=True) for j in range(4)]:
            em()

    nc.compile()
    return nc


def _round_f32r(a):
    """Round fp32 array to the PE's FP32R format (RNE at 12 low mantissa bits)."""
    u = np.ascontiguousarray(a, np.float32).view(np.uint32).astype(np.uint64)
    low = u & 0xFFF
    up = (low > 0x800) | ((low == 0x800) & (((u >> 12) & 1) == 1))
    r = (u & ~np.uint64(0xFFF)) + np.where(up, 0x1000, 0)
    return r.astype(np.uint32).view(np.float32)


def _to_f16(a):
    return np.ascontiguousarray(a, np.float16)


def _rope_tables(s: int):
    inv_freq = 1.0 / (ROPE_THETA ** (np.arange(0, D_K, 2, dtype=np.float64) / D_K))
    angles = np.arange(s, dtype=np.float64)[:, None] * inv_freq[None, :]  # [s, 32]
    cos = np.cos(angles).astype(np.float32)  # [s, 32]
    sin = np.sin(angles).astype(np.float32)
    cosT = np.empty((D_K, s), np.float32)
    sinT = np.empty((D_K, s), np.float32)
    cosT[0::2] = cos.T
    cosT[1::2] = cos.T
    sinT[0::2] = -sin.T
    sinT[1::2] = sin.T
    return (
        np.ascontiguousarray(np.vstack([cosT, cosT])).astype(np.float16),
        np.ascontiguousarray(np.vstack([sinT, sinT])).astype(np.float16),
    )


def kernel(x, Wq, Wk, Wv, Wo, use_rope):
    from concourse.bass_utils import run_bass_kernel_spmd

    x = np.asarray(x, dtype=np.float32)
    ur = bool(int(np.asarray(use_rope)))
    key = (ur, S)
    if key not in _PROGRAM_CACHE:
        _PROGRAM_CACHE[key] = _build_program(ur, S)
    nc = _PROGRAM_CACHE[key]

    if ur:
        cosT, sinT = _rope_tables(S)

    in_maps = []
    for c in range(N_CORES):
        b, hg = c // 2, c % 2
        sl = slice(hg * HG_FEATS, (hg + 1) * HG_FEATS)
        cv = _round_f32r if MM_DTYPE == "f32r" else _to_f16
        m = {
            "xT": cv(x[b].T),
            "wqT": cv(np.asarray(Wq, np.float32)[sl, :].T),
            "wkT": cv(np.asarray(Wk, np.float32)[sl, :].T),
            "wvT": cv(np.asarray(Wv, np.float32)[sl, :].T),
            "woT": cv(np.asarray(Wo, np.float32)[:, sl].T),
        }
        if ur:
            m["cosT"] = cosT
            m["sinT"] = sinT
        in_maps.append(m)

    res = run_bass_kernel_spmd(nc, in_maps, list(range(N_CORES)))
    out = np.empty((B, S, D_MODEL), np.float32)
    for b in range(B):
        if K2_WOPAIR:
            out[b] = (
                res.results[2 * b]["out_a"].astype(np.float32)
                + res.results[2 * b]["out_b"].astype(np.float32)
                + res.results[2 * b + 1]["out_a"].astype(np.float32)
                + res.results[2 * b + 1]["out_b"].astype(np.float32)
            )
        else:
            out[b] = res.results[2 * b]["out"] + res.results[2 * b + 1]["out"]
    return out


# revision 5
# speedup vs baseline: 1.0258x; 1.0156x over previous
"""Multi-head self-attention (B=4, S=2048, D=1024, H=16, causal + RoPE) on 8
Trainium2 NeuronCores.  v2: PE tile-pair concurrency + causal trims.

Sharding: core c = (batch b = c // 2, head-group hg = c % 2).  Host sums the
two partial outputs per batch.

Device program (fp16 matmul operands, fp32 PSUM accumulation):
  - Scores emitted as adjacent row-tiled pairs (h0 rows 0-63, h1 rows 64-127,
    different PSUM banks) -> both heads stream concurrently (~2x).
  - Wo emitted as adjacent (h, half)-alternating pairs into two PSUM banks
    po_a (out cols 0-511) / po_b (cols 512-1023): concurrent, and no
    post-merge needed (each bank is a complete output half).
  - Diagonal key blocks: scores/attn@V/mask restricted to the valid query
    range (q >= 128*j within the chunk).
  - One exp per key-block-pair over [128, 2048] PSUM -> fewer ScalarE calls.
  - ctxu evacuation carries the Z row (f32, 65 partitions); Z reshape via
    DRAM roundtrip to [64, 16], reciprocal 64 lanes wide, broadcast-read DMA.
  - proj/Wo chains woven between attention kbp iterations to fill the PE
    while ScalarE exp paces the attention pipeline.
"""

import os
import numpy as np

K2_WEAVE = os.environ.get("K2_WEAVE", "1") == "1"
K2_QSPLIT = os.environ.get("K2_QSPLIT", "1") == "1"
K2_EXP2 = os.environ.get("K2_EXP2", "1") == "1"    # single [128,2048] exp call
K2_TRIM = os.environ.get("K2_TRIM", "1") == "1"    # attnV+mask q-trim on diagonal
K2_WOPAIR = os.environ.get("K2_WOPAIR", "1") == "1"  # Wo alternating po_a/po_b

D_MODEL = 1024
NUM_HEADS = 16
D_K = 64
ROPE_THETA = 10000.0
B = 4
S = 2048
N_CORES = 8

HG_FEATS = 512          # features per core (8 heads)
FT = HG_FEATS // 128    # head pairs per core
KT = D_MODEL // 128     # contraction tiles for the projections

MM_DTYPE = "f16"
_PROGRAM_CACHE = {}


def _build_program(use_rope: bool, s: int = S):
    import concourse.tile as tile
    from concourse import bacc, mybir
    from contextlib import ExitStack

    f32 = mybir.dt.float32
    f16 = mybir.dt.float16
    mmdt = f16
    FP = mybir.ActivationFunctionType

    SC = s // 512           # 512-wide seq chunks
    ST = s // 128           # 128-wide seq tiles
    PAIRSWAP = [i ^ 1 for i in range(32)]

    nc = bacc.Bacc("TRN2", target_bir_lowering=False, debug=False)

    xT = nc.dram_tensor("xT", [D_MODEL, s], mmdt, kind="ExternalInput")
    wqT = nc.dram_tensor("wqT", [D_MODEL, HG_FEATS], mmdt, kind="ExternalInput")
    wkT = nc.dram_tensor("wkT", [D_MODEL, HG_FEATS], mmdt, kind="ExternalInput")
    wvT = nc.dram_tensor("wvT", [D_MODEL, HG_FEATS], mmdt, kind="ExternalInput")
    woT = nc.dram_tensor("woT", [HG_FEATS, D_MODEL], mmdt, kind="ExternalInput")
    if use_rope:
        cosT = nc.dram_tensor("cosT", [128, s], f16, kind="ExternalInput")
        sinT = nc.dram_tensor("sinT", [128, s], f16, kind="ExternalInput")
    if K2_WOPAIR:
        out_a = nc.dram_tensor("out_a", [s, D_MODEL], f16, kind="ExternalOutput")
        out_b = nc.dram_tensor("out_b", [s, D_MODEL], f16, kind="ExternalOutput")
    else:
        out = nc.dram_tensor("out", [s, D_MODEL], f32, kind="ExternalOutput")

    with tile.TileContext(nc) as tc, ExitStack() as ctx:
        singles = ctx.enter_context(tc.tile_pool(name="singles", bufs=1))
        stripes = ctx.enter_context(tc.tile_pool(name="stripes", bufs=2))
        tmppool = ctx.enter_context(tc.tile_pool(name="tmppool", bufs=2))
        exppool = ctx.enter_context(tc.tile_pool(name="exppool", bufs=3))
        ctxpool = ctx.enter_context(tc.tile_pool(name="ctxpool", bufs=3))
        smallp = ctx.enter_context(tc.tile_pool(name="smallp", bufs=4))
        outpool = ctx.enter_context(tc.tile_pool(name="outpool", bufs=4))
        dramp = ctx.enter_context(tc.tile_pool(name="dramp", bufs=1, space="DRAM"))
        psum = ctx.enter_context(tc.tile_pool(name="psum", bufs=1, space="PSUM"))

        # ---- persistent tiles -------------------------------------------
        wq_full = singles.tile([128, KT, HG_FEATS], mmdt, tag="wqf")
        wk_full = singles.tile([128, KT, HG_FEATS], mmdt, tag="wkf")
        wv_full = singles.tile([128, KT, HG_FEATS], mmdt, tag="wvf")
        wo_sb = singles.tile([128, FT, D_MODEL], mmdt, tag="wo")

        # Initial loads split across HWDGE queues: sync carries what the
        # first V matmuls need (wv + stripe 0); vector carries wq/wk; scalar
        # carries wo (+ rope tables).
        nc.sync.dma_start(
            out=wv_full[:, 0 : KT // 2, :],
            in_=wvT.ap()[0 : D_MODEL // 2, :].rearrange("(k p) f -> p k f", p=128),
        )
        sh_first = stripes.tile([128, KT, 512], mmdt, tag="stripe", name="sh0")
        for w in range(2):
            ks = slice(w * KT // 2, (w + 1) * KT // 2)
            nc.sync.dma_start(
                out=sh_first[:, ks, :],
                in_=xT.ap()[(ks.start * 128) : (ks.stop * 128), 0:512].rearrange(
                    "(k p) s -> p k s", p=128
                ),
            )
        nc.sync.dma_start(
            out=wv_full[:, KT // 2 : KT, :],
            in_=wvT.ap()[D_MODEL // 2 :, :].rearrange("(k p) f -> p k f", p=128),
        )
        qldma = nc.scalar if K2_QSPLIT else nc.sync
        for wsb, wdr in ((wq_full, wqT), (wk_full, wkT)):
            qldma.dma_start(
                out=wsb, in_=wdr.ap().rearrange("(k p) f -> p k f", p=128)
            )
        if use_rope:
            cos_sb = singles.tile([128, s], f16, tag="cos")
            sin_sb = singles.tile([128, s], f16, tag="sin")
            qldma.dma_start(out=cos_sb, in_=cosT.ap())
            qldma.dma_start(out=sin_sb, in_=sinT.ap())
        _wo_load = [
            lambda: qldma.dma_start(
                out=wo_sb, in_=woT.ap().rearrange("(f p) o -> p f o", p=128)
            )
        ]

        # V resident in SBUF: per (kb, hp) cols [V_h0(64) | 1 | pad | V_h1(64) | 1 | pad]
        vres = singles.tile([128, ST, FT, 132], mmdt, tag="vres")
        nc.vector.memset(
            vres[:].rearrange("p st hp (h c) -> p (st hp h) c", c=66)[:, :, 64:65],
            1.0,
        )
        # Z rows roundtrip through DRAM: [1, 1024] -> [64, 16] for a 64-lane
        # reciprocal, then 1/Z broadcast back across 64 partitions.
        ztmp = dramp.tile([FT, SC, 1, 1024], f32, tag="ztmp")
        ztmp2 = dramp.tile([FT, SC, 1, 1024], f16, tag="ztmp2")
        QTs = [
            singles.tile([128, s], mmdt, tag=f"QT{hp}", name=f"QT{hp}")
            for hp in range(FT)
        ]
        KTs = [
            singles.tile([128, s], mmdt, tag=f"KT{hp}", name=f"KT{hp}")
            for hp in range(FT)
        ]

        def load_stripe(sc):
            sh = stripes.tile([128, KT, 512], mmdt, tag="stripe", name=f"sh{sc}")
            nc.sync.dma_start(
                out=sh,
                in_=xT.ap()[:, sc * 512 : (sc + 1) * 512].rearrange(
                    "(k p) s -> p k s", p=128
                ),
            )
            return sh

        # ---- work chains (each closure emits ~1-2us of PE work) ----------
        def vproj_chain(sc, sh, j):
            def emit():
                st = sc * 4 + j
                pv = psum.tile([128, 512], f32, tag="pqk", bufs=2, name="pv")
                for kt in range(KT):
                    nc.tensor.matmul(
                        pv,
                        lhsT=sh[:, kt, j * 128 : (j + 1) * 128],
                        rhs=wv_full[:, kt, :],
                        start=(kt == 0),
                        stop=(kt == KT - 1),
                    )
                # scatter into vres: dest inner 64 f16 contiguous, 132-col
                # blocks keep h-offsets 4B-aligned (DVE evacuation)
                nc.vector.tensor_copy(
                    vres[:, st].rearrange("p hp (h c) -> p (hp h) c", c=66)[
                        :, :, 0:64
                    ],
                    pv[:].rearrange("p (x c) -> p x c", c=64),
                )
            return emit

        def qkproj_chain(sc, sh, hp, which):
            def emit():
                w_sb = wq_full if which == 0 else wk_full
                dst = QTs[hp] if which == 0 else KTs[hp]
                pq = psum.tile([128, 512], f32, tag="pqk", bufs=2, name="pq")
                for kt in range(KT):
                    nc.tensor.matmul(
                        pq,
                        lhsT=w_sb[:, kt, hp * 128 : (hp + 1) * 128],
                        rhs=sh[:, kt, :],
                        start=(kt == 0),
                        stop=(kt == KT - 1),
                    )
                dcol = dst[:, sc * 512 : (sc + 1) * 512]
                if use_rope:
                    ccol = cos_sb[:, sc * 512 : (sc + 1) * 512]
                    scol = sin_sb[:, sc * 512 : (sc + 1) * 512]
                    qsb = tmppool.tile([128, 512], f16, tag="qsb")
                    qcos = tmppool.tile([128, 512], f16, tag="qcos")
                    rot = tmppool.tile([128, 512], f16, tag="rot")
                    nc.scalar.activation(qsb, pq, FP.Copy)
                    nc.vector.stream_shuffle(rot, qsb, PAIRSWAP)
                    nc.vector.tensor_mul(qcos, qsb, ccol)
                    nc.vector.tensor_mul(rot, rot, scol)
                    nc.vector.tensor_add(dcol, qcos, rot)
                else:
                    nc.vector.tensor_copy(dcol, pq)
            return emit

        def wo_chain_base(qc, ctxn, j):
            def emit():
                st = qc * 4 + j
                osb = outpool.tile([128, 1024], f32, tag="osb")
                for half in range(2):
                    po = [
                        psum.tile([128, 512], f32, tag="pqk", bufs=2, name=f"po{h}")
                        for h in range(2)
                    ]
                    for hp in range(FT):
                        for h in range(2):
                            nc.tensor.matmul(
                                po[h],
                                lhsT=ctxn[hp][
                                    64 * h : 64 * h + 64, j * 128 : (j + 1) * 128
                                ],
                                rhs=wo_sb[
                                    64 * h : 64 * h + 64,
                                    hp,
                                    half * 512 : (half + 1) * 512,
                                ],
                                start=(hp == 0),
                                stop=(hp == FT - 1),
                                skip_group_check=True,
                            )
                    oh = osb[:, half * 512 : (half + 1) * 512]
                    nc.scalar.activation(oh, po[0], FP.Copy)
                    nc.vector.scalar_tensor_tensor(
                        out=oh,
                        in0=po[1],
                        scalar=1.0,
                        in1=oh,
                        op0=mybir.AluOpType.mult,
                        op1=mybir.AluOpType.add,
                    )
                nc.sync.dma_start(
                    out=out.ap()[st * 128 : (st + 1) * 128, :], in_=osb
                )
            return emit

        def wo_chain(qc, ctxn, j, epilogue=False):
            if not K2_WOPAIR:
                return wo_chain_base(qc, ctxn, j)

            def emit():
                st = qc * 4 + j
                # 4 chains, each with a consistent PE row group:
                #   h0 chains (rows 0-63)  -> osb_a -> out_a
                #   h1 chains (rows 64-127) -> osb_b -> out_b
                # (h0, half) and (h1, half) emitted adjacently -> concurrent.
                osb_a = outpool.tile([128, 1024], f16, tag="osba")
                osb_b = outpool.tile([128, 1024], f16, tag="osbb")
                for half in range(2):
                    if epilogue and (2 * j + half) % 2 == 1:
                        pop = psum.tile(
                            [128, 1024], f32, tag="oacc", bufs=1, name="pop"
                        )
                        po0 = pop[:, 0:512]
                        po1 = pop[:, 512:1024]
                    else:
                        po0 = psum.tile([128, 512], f32, tag="pqk", bufs=2, name="po0")
                        po1 = psum.tile([128, 512], f32, tag="pqk", bufs=2, name="po1")
                    for hp in range(FT):
                        for h, po in ((0, po0), (1, po1)):
                            nc.tensor.matmul(
                                po,
                                lhsT=ctxn[hp][
                                    64 * h : 64 * h + 64, j * 128 : (j + 1) * 128
                                ],
                                rhs=wo_sb[
                                    64 * h : 64 * h + 64,
                                    hp,
                                    half * 512 : (half + 1) * 512,
                                ],
                                start=(hp == 0),
                                stop=(hp == FT - 1),
                                skip_group_check=True,
                            )
                    # evacuate: one on ScalarE, one on DVE (engine balance)
                    nc.scalar.activation(
                        osb_a[:, half * 512 : (half + 1) * 512], po0, FP.Copy
                    )
                    nc.vector.tensor_copy(
                        osb_b[:, half * 512 : (half + 1) * 512], po1
                    )
                nc.sync.dma_start(
                    out=out_a.ap()[st * 128 : (st + 1) * 128, :], in_=osb_a
                )
                nc.sync.dma_start(
                    out=out_b.ap()[st * 128 : (st + 1) * 128, :], in_=osb_b
                )
            return emit

        # ---- attention ---------------------------------------------------
        def do_attn(qc, work):
            nkb = 4 * qc + 4
            ctxn = [
                ctxpool.tile([128, 512], f16, tag=f"ctxn{hp}", name=f"ctxn{hp}")
                for hp in range(FT)
            ]
            for hp in range(FT):
                QT = QTs[hp]
                KTt = KTs[hp]
                oacc = psum.tile([128, 1024], f32, tag="oacc", bufs=1, name="oacc")
                def emit_attnv(kb, expt):
                    j = kb - 4 * qc
                    q0 = 128 * j if (j >= 0 and K2_TRIM) else 0
                    for h in range(2):
                        nc.tensor.matmul(
                            oacc[0:65, 512 * h + q0 : 512 * h + 512],
                            lhsT=vres[:, kb, hp, 66 * h : 66 * h + 65],
                            rhs=expt[:, 512 * h + q0 : 512 * h + 512],
                            start=(kb == 0),
                            stop=(kb == nkb - 1),
                            skip_group_check=True,
                        )

                prev = None
                for kb in range(nkb):
                    if kb % 2 == 0:
                        _drain_due(work, (qc, hp, kb // 2))
                    # scores [128, 1024]: [h0 | h1], double-buffered
                    scr = psum.tile(
                        [128, 1024], f32, tag="scores", bufs=2, name="scores"
                    )
                    for h in range(2):
                        nc.tensor.matmul(
                            scr[:, 512 * h : 512 * h + 512],
                            lhsT=KTt[
                                64 * h : 64 * h + 64, kb * 128 : (kb + 1) * 128
                            ],
                            rhs=QT[
                                64 * h : 64 * h + 64,
                                qc * 512 : (qc + 1) * 512,
                            ],
                            start=True,
                            stop=True,
                            skip_group_check=True,
                        )
                    expt = exppool.tile([128, 1024], mmdt, tag="expt")
                    nc.scalar.activation(expt, scr, FP.Exp, scale=0.125)
                    j = kb - 4 * qc
                    if j >= 0:  # diagonal tile: causal mask post-exp
                        q0 = 128 * j if K2_TRIM else 0
                        for h in range(2):
                            sl = expt[:, 512 * h + q0 : 512 * h + 512]
                            nc.gpsimd.affine_select(
                                out=sl,
                                in_=sl,
                                compare_op=mybir.AluOpType.is_ge,
                                fill=0.0,
                                base=-(128 * j - q0),
                                pattern=[[1, 512 - q0]],
                                channel_multiplier=-1,
                            )
                    # previous kb's attn@V (exp/mask latency covered)
                    if prev is not None:
                        emit_attnv(*prev)
                    prev = (kb, expt)
                emit_attnv(*prev)
                # ---- evacuate ctx+Z, normalize -------------------------
                ctxu = smallp.tile([128, 1024], f32, tag="ctxu")
                nc.vector.tensor_copy(ctxu[0:65, :], oacc[0:65, :])
                dmae = nc.sync
                dmae.dma_start(out=ztmp[hp, qc], in_=ctxu[64:65, :])
                zT = smallp.tile([64, 16], f32, tag="zT")
                dmae.dma_start(
                    out=zT, in_=ztmp[hp, qc, 0].rearrange("(p g) -> p g", p=64)
                )
                zinvT = smallp.tile([64, 16], f32, tag="zinvT")
                nc.vector.reciprocal(zinvT, zT)
                zinvT16 = smallp.tile([64, 16], f16, tag="zinvT16")
                nc.vector.tensor_copy(zinvT16, zinvT)
                dmae.dma_start(
                    out=ztmp2[hp, qc, 0].rearrange("(p g) -> p g", p=64),
                    in_=zinvT16,
                )
                zbc_sb = smallp.tile([128, 1024], f16, tag="zbcsb")
                dmae.dma_start(
                    out=zbc_sb[0:64, :],
                    in_=ztmp2[hp, qc].broadcast_to([64, 1024]),
                )
                for h in range(2):
                    nc.vector.scalar_tensor_tensor(
                        out=ctxn[hp][64 * h : 64 * h + 64, :],
                        in0=ctxu[0:64, 512 * h : 512 * h + 512],
                        scalar=1.0,
                        in1=zbc_sb[0:64, 512 * h : 512 * h + 512],
                        op0=mybir.AluOpType.mult,
                        op1=mybir.AluOpType.mult,
                    )
            return ctxn

        # ---- deadline-scheduled emission ---------------------------------
        # Every chain gets an emission deadline (qc, hp, kbp): it is emitted
        # just before that attention step.  Deadlines place each chain where
        # the PE needs fill (late chunks have more exp time to cover), always
        # at-or-before the step that first reads the chain's outputs.
        work = []

        def _drain_due(work, step):
            i = 0
            while i < len(work):
                dl, em = work[i]
                if dl <= step:
                    em()
                    work.pop(i)
                else:
                    i += 1

        # proj(0): V j0/j1 + QK hp0 land in the prologue (deadline (0,0,0));
        # the rest weave into attn(0) at their first-read step.
        PROJ_DL = {
            0: {("v", 0): (0, 0, 0), ("v", 1): (0, 0, 0),
                ("v", 2): (0, 0, 1), ("v", 3): (0, 0, 1),
                ("qk", 0, 0): (0, 0, 0), ("qk", 0, 1): (0, 0, 0),
                ("qk", 1, 0): (0, 1, 0), ("qk", 1, 1): (0, 1, 0),
                ("qk", 2, 0): (0, 2, 0), ("qk", 2, 1): (0, 2, 0),
                ("qk", 3, 0): (0, 3, 0), ("qk", 3, 1): (0, 3, 0)},
            # proj(1): V + QK(hp0) inside attn(0); the rest early in attn(1)
            1: {("v", 0): (0, 0, 1), ("v", 1): (0, 1, 1),
                ("v", 2): (0, 2, 1), ("v", 3): (0, 3, 1),
                ("qk", 0, 0): (0, 1, 1), ("qk", 0, 1): (0, 2, 1),
                ("qk", 1, 0): (1, 0, 0), ("qk", 1, 1): (1, 0, 1),
                ("qk", 2, 0): (1, 1, 0), ("qk", 2, 1): (1, 1, 1),
                ("qk", 3, 0): (1, 2, 0), ("qk", 3, 1): (1, 2, 1)},
            # proj(2): spread across attn(1)
            2: {("v", 0): (1, 0, 1), ("v", 1): (1, 1, 1),
                ("v", 2): (1, 2, 1), ("v", 3): (1, 3, 1),
                ("qk", 0, 0): (1, 0, 2), ("qk", 0, 1): (1, 0, 3),
                ("qk", 1, 0): (1, 1, 2), ("qk", 1, 1): (1, 1, 3),
                ("qk", 2, 0): (1, 2, 2), ("qk", 2, 1): (1, 2, 3),
                ("qk", 3, 0): (1, 3, 2), ("qk", 3, 1): (1, 3, 3)},
            # proj(3): V + QK(hp0) spread across attn(2); QK(hp1-3) spill
            # into attn(3) just before each hp needs them
            3: {("v", 0): (2, 0, 2), ("v", 1): (2, 0, 5),
                ("v", 2): (2, 1, 1), ("v", 3): (2, 1, 4),
                ("qk", 0, 0): (2, 0, 0), ("qk", 0, 1): (2, 2, 0),
                ("qk", 1, 0): (3, 0, 1), ("qk", 1, 1): (3, 0, 4),
                ("qk", 2, 0): (3, 1, 1), ("qk", 2, 1): (3, 1, 5),
                ("qk", 3, 0): (3, 2, 2), ("qk", 3, 1): (3, 2, 4)},
        }
        # wo(sc) spread: wo(0) into attn(2); wo(1)/wo(2) into attn(3)
        WO_DL = {
            0: [(2, 2, 2), (2, 2, 5), (2, 3, 1), (2, 3, 3)],
            1: [(3, 0, 6), (3, 1, 3), (3, 1, 7), (3, 2, 6)],
            # wo(2): no in-loop deadline -> emitted at the post-loop flush,
            # leaving ready PE work to cover the last Z-chain latency
            2: [(3, 99, 0), (3, 99, 0), (3, 99, 0), (3, 99, 0)],
        }

        def add_proj_work(sc, sh):
            dls = PROJ_DL[sc]
            for j in range(4):
                work.append((dls[("v", j)], vproj_chain(sc, sh, j)))
            for hp in range(FT):
                for w in (0, 1):
                    work.append((dls[("qk", hp, w)], qkproj_chain(sc, sh, hp, w)))

        def warmfill():
            # HAM warm-keeper: dummy MMs gated on the last wv piece so they
            # fill the early DMA-starvation gap; results are overwritten by
            # the first real scores (start=True clears the banks).
            scr = psum.tile([128, 1024], f32, tag="scores", bufs=2, name="warm")
            for i in range(8):
                nc.tensor.matmul(
                    scr[:, 512 * (i % 2) : 512 * (i % 2) + 512],
                    lhsT=wv_full[:, KT - 1, (i % 4) * 128 : (i % 4) * 128 + 128],
                    rhs=wv_full[:, KT - 2, 0:512],
                    start=True,
                    stop=True,
                    skip_group_check=True,
                )

        add_proj_work(0, sh_first)
        work.append(((0, 0, 0), warmfill))
        work.append(((0, 2, 0), _wo_load[0]))
        ctxs = {}
        for sc in range(SC):
            if sc + 1 < SC:
                sh = load_stripe(sc + 1)
                add_proj_work(sc + 1, sh)
            if sc - 1 >= 0 and sc - 1 in WO_DL:
                prev_ctx = ctxs[sc - 1]
                for j in range(4):
                    work.append((WO_DL[sc - 1][j], wo_chain(sc - 1, prev_ctx, j)))
            ctxs[sc] = do_attn(sc, work)
        for dl, em in work:  # anything left (defensive)
            em()
        work.clear()
        last_ctx = ctxs[SC - 1]
        for em in [wo_chain(SC - 1, last_ctx, j, epilogue=True) for j in range(4)]:
            em()

    nc.compile()
    return nc


def _round_f32r(a):
    """Round fp32 array to the PE's FP32R format (RNE at 12 low mantissa bits)."""
    u = np.ascontiguousarray(a, np.float32).view(np.uint32).astype(np.uint64)
    low = u & 0xFFF
    up = (low > 0x800) | ((low == 0x800) & (((u >> 12) & 1) == 1))
    r = (u & ~np.uint64(0xFFF)) + np.where(up, 0x1000, 0)
    return r.astype(np.uint32).view(np.float32)


def _to_f16(a):
    return np.ascontiguousarray(a, np.float16)


def _rope_tables(s: int):
    inv_freq = 1.0 / (ROPE_THETA ** (np.arange(0, D_K, 2, dtype=np.float64) / D_K))
    angles = np.arange(s, dtype=np.float64)[:, None] * inv_freq[None, :]  # [s, 32]
    cos = np.cos(angles).astype(np.float32)  # [s, 32]
    sin = np.sin(angles).astype(np.float32)
    cosT = np.empty((D_K, s), np.float32)
    sinT = np.empty((D_K, s), np.float32)
    cosT[0::2] = cos.T
    cosT[1::2] = cos.T
    sinT[0::2] = -sin.T
    sinT[1::2] = sin.T
    return (
        np.ascontiguousarray(np.vstack([cosT, cosT])).astype(np.float16),
        np.ascontiguousarray(np.vstack([sinT, sinT])).astype(np.float16),
    )


def kernel(x, Wq, Wk, Wv, Wo, use_rope):
    from concourse.bass_utils import run_bass_kernel_spmd

    x = np.asarray(x, dtype=np.float32)
    ur = bool(int(np.asarray(use_rope)))
    key = (ur, S)
    if key not in _PROGRAM_CACHE:
        _PROGRAM_CACHE[key] = _build_program(ur, S)
    nc = _PROGRAM_CACHE[key]

    if ur:
        cosT, sinT = _rope_tables(S)

    in_maps = []
    for c in range(N_CORES):
        b, hg = c // 2, c % 2
        sl = slice(hg * HG_FEATS, (hg + 1) * HG_FEATS)
        cv = _round_f32r if MM_DTYPE == "f32r" else _to_f16
        m = {
            "xT": cv(x[b].T),
            "wqT": cv(np.asarray(Wq, np.float32)[sl, :].T),
            "wkT": cv(np.asarray(Wk, np.float32)[sl, :].T),
            "wvT": cv(np.asarray(Wv, np.float32)[sl, :].T),
            "woT": cv(np.asarray(Wo, np.float32)[:, sl].T),
        }
        if ur:
            m["cosT"] = cosT
            m["sinT"] = sinT
        in_maps.append(m)

    res = run_bass_kernel_spmd(nc, in_maps, list(range(N_CORES)))
    out = np.empty((B, S, D_MODEL), np.float32)
    for b in range(B):
        if K2_WOPAIR:
            out[b] = (
                res.results[2 * b]["out_a"].astype(np.float32)
                + res.results[2 * b]["out_b"].astype(np.float32)
                + res.results[2 * b + 1]["out_a"].astype(np.float32)
                + res.results[2 * b + 1]["out_b"].astype(np.float32)
            )
        else:
            out[b] = res.results[2 * b]["out"] + res.results[2 * b + 1]["out"]
    return out


# revision 6
# speedup vs baseline: 1.0593x; 1.0326x over previous
"""Multi-head self-attention (B=4, S=2048, D=1024, H=16, causal + RoPE) on 8
Trainium2 NeuronCores.  v2: PE tile-pair concurrency + causal trims.

Sharding: core c = (batch b = c // 2, head-group hg = c % 2).  Host sums the
two partial outputs per batch.

Device program (fp16 matmul operands, fp32 PSUM accumulation):
  - Scores emitted as adjacent row-tiled pairs (h0 rows 0-63, h1 rows 64-127,
    different PSUM banks) -> both heads stream concurrently (~2x).
  - Wo emitted as adjacent (h, half)-alternating pairs into two PSUM banks
    po_a (out cols 0-511) / po_b (cols 512-1023): concurrent, and no
    post-merge needed (each bank is a complete output half).
  - Diagonal key blocks: scores/attn@V/mask restricted to the valid query
    range (q >= 128*j within the chunk).
  - One exp per key-block-pair over [128, 2048] PSUM -> fewer ScalarE calls.
  - ctxu evacuation carries the Z row (f32, 65 partitions); Z reshape via
    DRAM roundtrip to [64, 16], reciprocal 64 lanes wide, broadcast-read DMA.
  - proj/Wo chains woven between attention kbp iterations to fill the PE
    while ScalarE exp paces the attention pipeline.
"""

import os
import numpy as np

K2_WEAVE = os.environ.get("K2_WEAVE", "1") == "1"
K2_QSPLIT = os.environ.get("K2_QSPLIT", "1") == "1"
K2_EXP2 = os.environ.get("K2_EXP2", "1") == "1"    # single [128,2048] exp call
K2_TRIM = os.environ.get("K2_TRIM", "1") == "1"    # attnV+mask q-trim on diagonal
K2_WOPAIR = os.environ.get("K2_WOPAIR", "1") == "1"  # Wo alternating po_a/po_b

D_MODEL = 1024
NUM_HEADS = 16
D_K = 64
ROPE_THETA = 10000.0
B = 4
S = 2048
N_CORES = 8

HG_FEATS = 512          # features per core (8 heads)
FT = HG_FEATS // 128    # head pairs per core
KT = D_MODEL // 128     # contraction tiles for the projections

MM_DTYPE = "f16"
_PROGRAM_CACHE = {}


def _build_program(use_rope: bool, s: int = S):
    import concourse.tile as tile
    from concourse import bacc, mybir
    from contextlib import ExitStack

    f32 = mybir.dt.float32
    f16 = mybir.dt.float16
    mmdt = f16
    FP = mybir.ActivationFunctionType

    SC = s // 512           # 512-wide seq chunks
    ST = s // 128           # 128-wide seq tiles
    PAIRSWAP = [i ^ 1 for i in range(32)]

    nc = bacc.Bacc("TRN2", target_bir_lowering=False, debug=False)

    xT = nc.dram_tensor("xT", [D_MODEL, s], mmdt, kind="ExternalInput")
    wqT = nc.dram_tensor("wqT", [D_MODEL, HG_FEATS], mmdt, kind="ExternalInput")
    wkT = nc.dram_tensor("wkT", [D_MODEL, HG_FEATS], mmdt, kind="ExternalInput")
    wvT = nc.dram_tensor("wvT", [D_MODEL, HG_FEATS], mmdt, kind="ExternalInput")
    woT = nc.dram_tensor("woT", [HG_FEATS, D_MODEL], mmdt, kind="ExternalInput")
    if use_rope:
        cosT = nc.dram_tensor("cosT", [128, s], f16, kind="ExternalInput")
        sinT = nc.dram_tensor("sinT", [128, s], f16, kind="ExternalInput")
    if K2_WOPAIR:
        out_a = nc.dram_tensor("out_a", [s, D_MODEL], f16, kind="ExternalOutput")
        out_b = nc.dram_tensor("out_b", [s, D_MODEL], f16, kind="ExternalOutput")
    else:
        out = nc.dram_tensor("out", [s, D_MODEL], f32, kind="ExternalOutput")

    with tile.TileContext(nc) as tc, ExitStack() as ctx:
        singles = ctx.enter_context(tc.tile_pool(name="singles", bufs=1))
        stripes = ctx.enter_context(tc.tile_pool(name="stripes", bufs=2))
        tmppool = ctx.enter_context(tc.tile_pool(name="tmppool", bufs=2))
        exppool = ctx.enter_context(tc.tile_pool(name="exppool", bufs=3))
        ctxpool = ctx.enter_context(tc.tile_pool(name="ctxpool", bufs=3))
        smallp = ctx.enter_context(tc.tile_pool(name="smallp", bufs=4))
        outpool = ctx.enter_context(tc.tile_pool(name="outpool", bufs=4))
        dramp = ctx.enter_context(tc.tile_pool(name="dramp", bufs=1, space="DRAM"))
        psum = ctx.enter_context(tc.tile_pool(name="psum", bufs=1, space="PSUM"))

        # ---- persistent tiles -------------------------------------------
        wq_full = singles.tile([128, KT, HG_FEATS], mmdt, tag="wqf")
        wk_full = singles.tile([128, KT, HG_FEATS], mmdt, tag="wkf")
        wv_full = singles.tile([128, KT, HG_FEATS], mmdt, tag="wvf")
        wo_sb = singles.tile([128, FT, D_MODEL], mmdt, tag="wo")

        # Initial loads split across HWDGE queues: sync carries what the
        # first V matmuls need (wv + stripe 0); vector carries wq/wk; scalar
        # carries wo (+ rope tables).
        nc.sync.dma_start(
            out=wv_full[:, 0 : KT // 2, :],
            in_=wvT.ap()[0 : D_MODEL // 2, :].rearrange("(k p) f -> p k f", p=128),
        )
        sh_first = stripes.tile([128, KT, 512], mmdt, tag="stripe", name="sh0")
        for w in range(2):
            ks = slice(w * KT // 2, (w + 1) * KT // 2)
            nc.sync.dma_start(
                out=sh_first[:, ks, :],
                in_=xT.ap()[(ks.start * 128) : (ks.stop * 128), 0:512].rearrange(
                    "(k p) s -> p k s", p=128
                ),
            )
        nc.sync.dma_start(
            out=wv_full[:, KT // 2 : KT, :],
            in_=wvT.ap()[D_MODEL // 2 :, :].rearrange("(k p) f -> p k f", p=128),
        )
        qldma = nc.scalar if K2_QSPLIT else nc.sync
        for wsb, wdr in ((wq_full, wqT), (wk_full, wkT)):
            qldma.dma_start(
                out=wsb, in_=wdr.ap().rearrange("(k p) f -> p k f", p=128)
            )
        if use_rope:
            cos_sb = singles.tile([128, s], f16, tag="cos")
            sin_sb = singles.tile([128, s], f16, tag="sin")
            qldma.dma_start(out=cos_sb, in_=cosT.ap())
            qldma.dma_start(out=sin_sb, in_=sinT.ap())
        _wo_load = [
            lambda: qldma.dma_start(
                out=wo_sb, in_=woT.ap().rearrange("(f p) o -> p f o", p=128)
            )
        ]

        # V resident in SBUF: per (kb, hp) cols [V_h0(64) | 1 | pad | V_h1(64) | 1 | pad]
        vres = singles.tile([128, ST, FT, 132], mmdt, tag="vres")
        nc.vector.memset(
            vres[:].rearrange("p st hp (h c) -> p (st hp h) c", c=66)[:, :, 64:65],
            1.0,
        )
        # Z rows roundtrip through DRAM: [1, 1024] -> [64, 16] for a 64-lane
        # reciprocal, then 1/Z broadcast back across 64 partitions.
        ztmp = dramp.tile([FT, SC, 1, 1024], f32, tag="ztmp")
        ztmp2 = dramp.tile([FT, SC, 1, 1024], f16, tag="ztmp2")
        QTs = [
            singles.tile([128, s], mmdt, tag=f"QT{hp}", name=f"QT{hp}")
            for hp in range(FT)
        ]
        KTs = [
            singles.tile([128, s], mmdt, tag=f"KT{hp}", name=f"KT{hp}")
            for hp in range(FT)
        ]

        def load_stripe(sc):
            sh = stripes.tile([128, KT, 512], mmdt, tag="stripe", name=f"sh{sc}")
            nc.sync.dma_start(
                out=sh,
                in_=xT.ap()[:, sc * 512 : (sc + 1) * 512].rearrange(
                    "(k p) s -> p k s", p=128
                ),
            )
            return sh

        # ---- work chains (each closure emits ~1-2us of PE work) ----------
        def vproj_chain(sc, sh, j):
            def emit():
                st = sc * 4 + j
                pv = psum.tile([128, 512], f32, tag="pqk", bufs=2, name="pv")
                for kt in range(KT):
                    nc.tensor.matmul(
                        pv,
                        lhsT=sh[:, kt, j * 128 : (j + 1) * 128],
                        rhs=wv_full[:, kt, :],
                        start=(kt == 0),
                        stop=(kt == KT - 1),
                    )
                # scatter into vres: dest inner 64 f16 contiguous, 132-col
                # blocks keep h-offsets 4B-aligned (DVE evacuation)
                nc.vector.tensor_copy(
                    vres[:, st].rearrange("p hp (h c) -> p (hp h) c", c=66)[
                        :, :, 0:64
                    ],
                    pv[:].rearrange("p (x c) -> p x c", c=64),
                )
            return emit

        def qkproj_chain(sc, sh, hp, which):
            def emit():
                w_sb = wq_full if which == 0 else wk_full
                dst = QTs[hp] if which == 0 else KTs[hp]
                pq = psum.tile([128, 512], f32, tag="pqk", bufs=2, name="pq")
                for kt in range(KT):
                    nc.tensor.matmul(
                        pq,
                        lhsT=w_sb[:, kt, hp * 128 : (hp + 1) * 128],
                        rhs=sh[:, kt, :],
                        start=(kt == 0),
                        stop=(kt == KT - 1),
                    )
                dcol = dst[:, sc * 512 : (sc + 1) * 512]
                if use_rope:
                    ccol = cos_sb[:, sc * 512 : (sc + 1) * 512]
                    scol = sin_sb[:, sc * 512 : (sc + 1) * 512]
                    qsb = tmppool.tile([128, 512], f16, tag="qsb")
                    qcos = tmppool.tile([128, 512], f16, tag="qcos")
                    rot = tmppool.tile([128, 512], f16, tag="rot")
                    nc.scalar.activation(qsb, pq, FP.Copy)
                    nc.vector.stream_shuffle(rot, qsb, PAIRSWAP)
                    nc.vector.tensor_mul(qcos, qsb, ccol)
                    nc.vector.tensor_mul(rot, rot, scol)
                    nc.vector.tensor_add(dcol, qcos, rot)
                else:
                    nc.vector.tensor_copy(dcol, pq)
            return emit

        def wo_chain_base(qc, ctxn, j):
            def emit():
                st = qc * 4 + j
                osb = outpool.tile([128, 1024], f32, tag="osb")
                for half in range(2):
                    po = [
                        psum.tile([128, 512], f32, tag="pqk", bufs=2, name=f"po{h}")
                        for h in range(2)
                    ]
                    for hp in range(FT):
                        for h in range(2):
                            nc.tensor.matmul(
                                po[h],
                                lhsT=ctxn[hp][
                                    64 * h : 64 * h + 64, j * 128 : (j + 1) * 128
                                ],
                                rhs=wo_sb[
                                    64 * h : 64 * h + 64,
                                    hp,
                                    half * 512 : (half + 1) * 512,
                                ],
                                start=(hp == 0),
                                stop=(hp == FT - 1),
                                skip_group_check=True,
                            )
                    oh = osb[:, half * 512 : (half + 1) * 512]
                    nc.scalar.activation(oh, po[0], FP.Copy)
                    nc.vector.scalar_tensor_tensor(
                        out=oh,
                        in0=po[1],
                        scalar=1.0,
                        in1=oh,
                        op0=mybir.AluOpType.mult,
                        op1=mybir.AluOpType.add,
                    )
                nc.sync.dma_start(
                    out=out.ap()[st * 128 : (st + 1) * 128, :], in_=osb
                )
            return emit

        def wo_chain(qc, ctxn, j, epilogue=False):
            if not K2_WOPAIR:
                return wo_chain_base(qc, ctxn, j)

            def emit():
                st = qc * 4 + j
                # 4 chains, each with a consistent PE row group:
                #   h0 chains (rows 0-63)  -> osb_a -> out_a
                #   h1 chains (rows 64-127) -> osb_b -> out_b
                # (h0, half) and (h1, half) emitted adjacently -> concurrent.
                osb_a = outpool.tile([128, 1024], f16, tag="osba")
                osb_b = outpool.tile([128, 1024], f16, tag="osbb")
                for half in range(2):
                    if epilogue and (2 * j + half) % 2 == 1:
                        pop = psum.tile(
                            [128, 1024], f32, tag="oacc", bufs=1, name="pop"
                        )
                        po0 = pop[:, 0:512]
                        po1 = pop[:, 512:1024]
                    else:
                        po0 = psum.tile([128, 512], f32, tag="pqk", bufs=2, name="po0")
                        po1 = psum.tile([128, 512], f32, tag="pqk", bufs=2, name="po1")
                    for hp in range(FT):
                        for h, po in ((0, po0), (1, po1)):
                            nc.tensor.matmul(
                                po,
                                lhsT=ctxn[hp][
                                    64 * h : 64 * h + 64, j * 128 : (j + 1) * 128
                                ],
                                rhs=wo_sb[
                                    64 * h : 64 * h + 64,
                                    hp,
                                    half * 512 : (half + 1) * 512,
                                ],
                                start=(hp == 0),
                                stop=(hp == FT - 1),
                                skip_group_check=True,
                            )
                    # evacuate: one on ScalarE, one on DVE (engine balance)
                    nc.scalar.activation(
                        osb_a[:, half * 512 : (half + 1) * 512], po0, FP.Copy
                    )
                    nc.vector.tensor_copy(
                        osb_b[:, half * 512 : (half + 1) * 512], po1
                    )
                nc.sync.dma_start(
                    out=out_a.ap()[st * 128 : (st + 1) * 128, :], in_=osb_a
                )
                nc.sync.dma_start(
                    out=out_b.ap()[st * 128 : (st + 1) * 128, :], in_=osb_b
                )
            return emit

        # ---- attention ---------------------------------------------------
        def do_attn(qc, work):
            nkb = 4 * qc + 4
            ctxn = [
                ctxpool.tile([128, 512], f16, tag=f"ctxn{hp}", name=f"ctxn{hp}")
                for hp in range(FT)
            ]
            for hp in range(FT):
                QT = QTs[hp]
                KTt = KTs[hp]
                oacc = psum.tile([128, 1024], f32, tag="oacc", bufs=1, name="oacc")
                def emit_attnv(kb, expt):
                    j = kb - 4 * qc
                    q0 = 128 * j if (j >= 0 and K2_TRIM) else 0
                    for h in range(2):
                        nc.tensor.matmul(
                            oacc[0:65, 512 * h + q0 : 512 * h + 512],
                            lhsT=vres[:, kb, hp, 66 * h : 66 * h + 65],
                            rhs=expt[:, 512 * h + q0 : 512 * h + 512],
                            start=(kb == 0),
                            stop=(kb == nkb - 1),
                            skip_group_check=True,
                        )

                prev = None
                for kb in range(nkb):
                    if kb % 2 == 0:
                        _drain_due(work, (qc, hp, kb // 2))
                    # scores [128, 1024]: [h0 | h1], double-buffered;
                    # diagonal blocks trim fully-masked query columns
                    j = kb - 4 * qc
                    q0 = 128 * j if (j >= 0 and K2_TRIM) else 0
                    scr = psum.tile(
                        [128, 1024], f32, tag="scores", bufs=2, name="scores"
                    )
                    for h in range(2):
                        nc.tensor.matmul(
                            scr[:, 512 * h + q0 : 512 * h + 512],
                            lhsT=KTt[
                                64 * h : 64 * h + 64, kb * 128 : (kb + 1) * 128
                            ],
                            rhs=QT[
                                64 * h : 64 * h + 64,
                                qc * 512 + q0 : (qc + 1) * 512,
                            ],
                            start=True,
                            stop=True,
                            skip_group_check=True,
                        )
                    expt = exppool.tile([128, 1024], mmdt, tag="expt")
                    if q0 == 0:
                        nc.scalar.activation(expt, scr, FP.Exp, scale=0.125)
                    else:
                        for h in range(2):
                            nc.scalar.activation(
                                expt[:, 512 * h + q0 : 512 * h + 512],
                                scr[:, 512 * h + q0 : 512 * h + 512],
                                FP.Exp,
                                scale=0.125,
                            )
                    if j >= 0:  # diagonal tile: causal mask post-exp
                        q0 = 128 * j if K2_TRIM else 0
                        for h in range(2):
                            sl = expt[:, 512 * h + q0 : 512 * h + 512]
                            nc.gpsimd.affine_select(
                                out=sl,
                                in_=sl,
                                compare_op=mybir.AluOpType.is_ge,
                                fill=0.0,
                                base=-(128 * j - q0),
                                pattern=[[1, 512 - q0]],
                                channel_multiplier=-1,
                            )
                    # previous kb's attn@V (exp/mask latency covered)
                    if prev is not None:
                        emit_attnv(*prev)
                    prev = (kb, expt)
                emit_attnv(*prev)
                # ---- evacuate ctx+Z, normalize -------------------------
                ctxu = smallp.tile([128, 1024], f32, tag="ctxu")
                nc.vector.tensor_copy(ctxu[0:65, :], oacc[0:65, :])
                dmae = nc.sync
                dmae.dma_start(out=ztmp[hp, qc], in_=ctxu[64:65, :])
                zT = smallp.tile([64, 16], f32, tag="zT")
                dmae.dma_start(
                    out=zT, in_=ztmp[hp, qc, 0].rearrange("(p g) -> p g", p=64)
                )
                zinvT = smallp.tile([64, 16], f32, tag="zinvT")
                nc.vector.reciprocal(zinvT, zT)
                zinvT16 = smallp.tile([64, 16], f16, tag="zinvT16")
                nc.vector.tensor_copy(zinvT16, zinvT)
                dmae.dma_start(
                    out=ztmp2[hp, qc, 0].rearrange("(p g) -> p g", p=64),
                    in_=zinvT16,
                )
                zbc_sb = smallp.tile([128, 1024], f16, tag="zbcsb")
                dmae.dma_start(
                    out=zbc_sb[0:64, :],
                    in_=ztmp2[hp, qc].broadcast_to([64, 1024]),
                )
                for h in range(2):
                    nc.vector.scalar_tensor_tensor(
                        out=ctxn[hp][64 * h : 64 * h + 64, :],
                        in0=ctxu[0:64, 512 * h : 512 * h + 512],
                        scalar=1.0,
                        in1=zbc_sb[0:64, 512 * h : 512 * h + 512],
                        op0=mybir.AluOpType.mult,
                        op1=mybir.AluOpType.mult,
                    )
            return ctxn

        # ---- deadline-scheduled emission ---------------------------------
        # Every chain gets an emission deadline (qc, hp, kbp): it is emitted
        # just before that attention step.  Deadlines place each chain where
        # the PE needs fill (late chunks have more exp time to cover), always
        # at-or-before the step that first reads the chain's outputs.
        work = []

        def _drain_due(work, step):
            i = 0
            while i < len(work):
                dl, em = work[i]
                if dl <= step:
                    em()
                    work.pop(i)
                else:
                    i += 1

        # proj(0): V j0/j1 + QK hp0 land in the prologue (deadline (0,0,0));
        # the rest weave into attn(0) at their first-read step.
        PROJ_DL = {
            0: {("v", 0): (0, 0, 0), ("v", 1): (0, 0, 0),
                ("v", 2): (0, 0, 1), ("v", 3): (0, 0, 1),
                ("qk", 0, 0): (0, 0, 0), ("qk", 0, 1): (0, 0, 0),
                ("qk", 1, 0): (0, 1, 0), ("qk", 1, 1): (0, 1, 0),
                ("qk", 2, 0): (0, 2, 0), ("qk", 2, 1): (0, 2, 0),
                ("qk", 3, 0): (0, 3, 0), ("qk", 3, 1): (0, 3, 0)},
            # proj(1): V + QK(hp0) inside attn(0); the rest early in attn(1)
            1: {("v", 0): (0, 0, 1), ("v", 1): (0, 1, 1),
                ("v", 2): (0, 2, 1), ("v", 3): (0, 3, 1),
                ("qk", 0, 0): (0, 1, 1), ("qk", 0, 1): (0, 2, 1),
                ("qk", 1, 0): (1, 0, 0), ("qk", 1, 1): (1, 0, 1),
                ("qk", 2, 0): (1, 1, 0), ("qk", 2, 1): (1, 1, 1),
                ("qk", 3, 0): (1, 2, 0), ("qk", 3, 1): (1, 2, 1)},
            # proj(2): spread across attn(1)
            2: {("v", 0): (1, 0, 1), ("v", 1): (1, 1, 1),
                ("v", 2): (1, 2, 1), ("v", 3): (1, 3, 1),
                ("qk", 0, 0): (1, 0, 2), ("qk", 0, 1): (1, 0, 3),
                ("qk", 1, 0): (1, 1, 2), ("qk", 1, 1): (1, 1, 3),
                ("qk", 2, 0): (1, 2, 2), ("qk", 2, 1): (1, 2, 3),
                ("qk", 3, 0): (1, 3, 2), ("qk", 3, 1): (1, 3, 3)},
            # proj(3): V + QK(hp0) spread across attn(2); QK(hp1-3) spill
            # into attn(3) just before each hp needs them
            3: {("v", 0): (2, 0, 2), ("v", 1): (2, 0, 5),
                ("v", 2): (2, 1, 1), ("v", 3): (2, 1, 4),
                ("qk", 0, 0): (2, 0, 0), ("qk", 0, 1): (2, 2, 0),
                ("qk", 1, 0): (3, 0, 1), ("qk", 1, 1): (3, 0, 4),
                ("qk", 2, 0): (3, 1, 1), ("qk", 2, 1): (3, 1, 5),
                ("qk", 3, 0): (3, 2, 2), ("qk", 3, 1): (3, 2, 4)},
        }
        # wo(sc) spread: wo(0) into attn(2); wo(1)/wo(2) into attn(3)
        WO_DL = {
            0: [(2, 2, 2), (2, 2, 5), (2, 3, 1), (2, 3, 3)],
            1: [(3, 0, 6), (3, 1, 3), (3, 1, 7), (3, 2, 6)],
            # wo(2): no in-loop deadline -> emitted at the post-loop flush,
            # leaving ready PE work to cover the last Z-chain latency
            2: [(3, 99, 0), (3, 99, 0), (3, 99, 0), (3, 99, 0)],
        }

        def add_proj_work(sc, sh):
            dls = PROJ_DL[sc]
            for j in range(4):
                work.append((dls[("v", j)], vproj_chain(sc, sh, j)))
            for hp in range(FT):
                for w in (0, 1):
                    work.append((dls[("qk", hp, w)], qkproj_chain(sc, sh, hp, w)))

        def warmfill():
            # HAM warm-keeper: dummy MMs gated on the last wv piece so they
            # fill the early DMA-starvation gap; results are overwritten by
            # the first real scores (start=True clears the banks).
            scr = psum.tile([128, 1024], f32, tag="scores", bufs=2, name="warm")
            for i in range(8):
                nc.tensor.matmul(
                    scr[:, 512 * (i % 2) : 512 * (i % 2) + 512],
                    lhsT=wv_full[:, KT - 1, (i % 4) * 128 : (i % 4) * 128 + 128],
                    rhs=wv_full[:, KT - 2, 0:512],
                    start=True,
                    stop=True,
                    skip_group_check=True,
                )

        add_proj_work(0, sh_first)
        work.append(((0, 0, 0), warmfill))
        work.append(((0, 2, 0), _wo_load[0]))
        ctxs = {}
        for sc in range(SC):
            if sc + 1 < SC:
                sh = load_stripe(sc + 1)
                add_proj_work(sc + 1, sh)
            if sc - 1 >= 0 and sc - 1 in WO_DL:
                prev_ctx = ctxs[sc - 1]
                for j in range(4):
                    work.append((WO_DL[sc - 1][j], wo_chain(sc - 1, prev_ctx, j)))
            ctxs[sc] = do_attn(sc, work)
        for dl, em in work:  # anything left (defensive)
            em()
        work.clear()
        last_ctx = ctxs[SC - 1]
        for em in [wo_chain(SC - 1, last_ctx, j, epilogue=True) for j in range(4)]:
            em()

    nc.compile()
    return nc


def _round_f32r(a):
    """Round fp32 array to the PE's FP32R format (RNE at 12 low mantissa bits)."""
    u = np.ascontiguousarray(a, np.float32).view(np.uint32).astype(np.uint64)
    low = u & 0xFFF
    up = (low > 0x800) | ((low == 0x800) & (((u >> 12) & 1) == 1))
    r = (u & ~np.uint64(0xFFF)) + np.where(up, 0x1000, 0)
    return r.astype(np.uint32).view(np.float32)


def _to_f16(a):
    return np.ascontiguousarray(a, np.float16)


def _rope_tables(s: int):
    inv_freq = 1.0 / (ROPE_THETA ** (np.arange(0, D_K, 2, dtype=np.float64) / D_K))
    angles = np.arange(s, dtype=np.float64)[:, None] * inv_freq[None, :]  # [s, 32]
    cos = np.cos(angles).astype(np.float32)  # [s, 32]
    sin = np.sin(angles).astype(np.float32)
    cosT = np.empty((D_K, s), np.float32)
    sinT = np.empty((D_K, s), np.float32)
    cosT[0::2] = cos.T
    cosT[1::2] = cos.T
    sinT[0::2] = -sin.T
    sinT[1::2] = sin.T
    return (
        np.ascontiguousarray(np.vstack([cosT, cosT])).astype(np.float16),
        np.ascontiguousarray(np.vstack([sinT, sinT])).astype(np.float16),
    )


def kernel(x, Wq, Wk, Wv, Wo, use_rope):
    from concourse.bass_utils import run_bass_kernel_spmd

    x = np.asarray(x, dtype=np.float32)
    ur = bool(int(np.asarray(use_rope)))
    key = (ur, S)
    if key not in _PROGRAM_CACHE:
        _PROGRAM_CACHE[key] = _build_program(ur, S)
    nc = _PROGRAM_CACHE[key]

    if ur:
        cosT, sinT = _rope_tables(S)

    in_maps = []
    for c in range(N_CORES):
        b, hg = c // 2, c % 2
        sl = slice(hg * HG_FEATS, (hg + 1) * HG_FEATS)
        cv = _round_f32r if MM_DTYPE == "f32r" else _to_f16
        m = {
            "xT": cv(x[b].T),
            "wqT": cv(np.asarray(Wq, np.float32)[sl, :].T),
            "wkT": cv(np.asarray(Wk, np.float32)[sl, :].T),
            "wvT": cv(np.asarray(Wv, np.float32)[sl, :].T),
            "woT": cv(np.asarray(Wo, np.float32)[:, sl].T),
        }
        if ur:
            m["cosT"] = cosT
            m["sinT"] = sinT
        in_maps.append(m)

    res = run_bass_kernel_spmd(nc, in_maps, list(range(N_CORES)))
    out = np.empty((B, S, D_MODEL), np.float32)
    for b in range(B):
        if K2_WOPAIR:
            out[b] = (
                res.results[2 * b]["out_a"].astype(np.float32)
                + res.results[2 * b]["out_b"].astype(np.float32)
                + res.results[2 * b + 1]["out_a"].astype(np.float32)
                + res.results[2 * b + 1]["out_b"].astype(np.float32)
            )
        else:
            out[b] = res.results[2 * b]["out"] + res.results[2 * b + 1]["out"]
    return out


# revision 7
# speedup vs baseline: 1.0646x; 1.0051x over previous
"""Multi-head self-attention (B=4, S=2048, D=1024, H=16, causal + RoPE) on 8
Trainium2 NeuronCores.  v2: PE tile-pair concurrency + causal trims.

Sharding: core c = (batch b = c // 2, head-group hg = c % 2).  Host sums the
two partial outputs per batch.

Device program (fp16 matmul operands, fp32 PSUM accumulation):
  - Scores emitted as adjacent row-tiled pairs (h0 rows 0-63, h1 rows 64-127,
    different PSUM banks) -> both heads stream concurrently (~2x).
  - Wo emitted as adjacent (h, half)-alternating pairs into two PSUM banks
    po_a (out cols 0-511) / po_b (cols 512-1023): concurrent, and no
    post-merge needed (each bank is a complete output half).
  - Diagonal key blocks: scores/attn@V/mask restricted to the valid query
    range (q >= 128*j within the chunk).
  - One exp per key-block-pair over [128, 2048] PSUM -> fewer ScalarE calls.
  - ctxu evacuation carries the Z row (f32, 65 partitions); Z reshape via
    DRAM roundtrip to [64, 16], reciprocal 64 lanes wide, broadcast-read DMA.
  - proj/Wo chains woven between attention kbp iterations to fill the PE
    while ScalarE exp paces the attention pipeline.
"""

import os
import numpy as np

K2_WEAVE = os.environ.get("K2_WEAVE", "1") == "1"
K2_QSPLIT = os.environ.get("K2_QSPLIT", "1") == "1"
K2_EXP2 = os.environ.get("K2_EXP2", "1") == "1"    # single [128,2048] exp call
K2_TRIM = os.environ.get("K2_TRIM", "1") == "1"    # attnV+mask q-trim on diagonal
K2_WOPAIR = os.environ.get("K2_WOPAIR", "1") == "1"  # Wo alternating po_a/po_b

D_MODEL = 1024
NUM_HEADS = 16
D_K = 64
ROPE_THETA = 10000.0
B = 4
S = 2048
N_CORES = 8

HG_FEATS = 512          # features per core (8 heads)
FT = HG_FEATS // 128    # head pairs per core
KT = D_MODEL // 128     # contraction tiles for the projections

MM_DTYPE = "f16"
_PROGRAM_CACHE = {}


def _build_program(use_rope: bool, s: int = S):
    import concourse.tile as tile
    from concourse import bacc, mybir
    from contextlib import ExitStack

    f32 = mybir.dt.float32
    f16 = mybir.dt.float16
    mmdt = f16
    FP = mybir.ActivationFunctionType

    SC = s // 512           # 512-wide seq chunks
    ST = s // 128           # 128-wide seq tiles
    PAIRSWAP = [i ^ 1 for i in range(32)]

    nc = bacc.Bacc("TRN2", target_bir_lowering=False, debug=False)

    xT = nc.dram_tensor("xT", [D_MODEL, s], mmdt, kind="ExternalInput")
    wqT = nc.dram_tensor("wqT", [D_MODEL, HG_FEATS], mmdt, kind="ExternalInput")
    wkT = nc.dram_tensor("wkT", [D_MODEL, HG_FEATS], mmdt, kind="ExternalInput")
    wvT = nc.dram_tensor("wvT", [D_MODEL, HG_FEATS], mmdt, kind="ExternalInput")
    woT = nc.dram_tensor("woT", [HG_FEATS, D_MODEL], mmdt, kind="ExternalInput")
    if use_rope:
        cosT = nc.dram_tensor("cosT", [128, s], f16, kind="ExternalInput")
        sinT = nc.dram_tensor("sinT", [128, s], f16, kind="ExternalInput")
    if K2_WOPAIR:
        out_a = nc.dram_tensor("out_a", [s, D_MODEL], f16, kind="ExternalOutput")
        out_b = nc.dram_tensor("out_b", [s, D_MODEL], f16, kind="ExternalOutput")
    else:
        out = nc.dram_tensor("out", [s, D_MODEL], f32, kind="ExternalOutput")

    with tile.TileContext(nc) as tc, ExitStack() as ctx:
        singles = ctx.enter_context(tc.tile_pool(name="singles", bufs=1))
        stripes = ctx.enter_context(tc.tile_pool(name="stripes", bufs=2))
        tmppool = ctx.enter_context(tc.tile_pool(name="tmppool", bufs=2))
        exppool = ctx.enter_context(tc.tile_pool(name="exppool", bufs=3))
        ctxpool = ctx.enter_context(tc.tile_pool(name="ctxpool", bufs=3))
        smallp = ctx.enter_context(tc.tile_pool(name="smallp", bufs=4))
        outpool = ctx.enter_context(tc.tile_pool(name="outpool", bufs=4))
        dramp = ctx.enter_context(tc.tile_pool(name="dramp", bufs=1, space="DRAM"))
        psum = ctx.enter_context(tc.tile_pool(name="psum", bufs=1, space="PSUM"))

        # ---- persistent tiles -------------------------------------------
        wq_full = singles.tile([128, KT, HG_FEATS], mmdt, tag="wqf")
        wk_full = singles.tile([128, KT, HG_FEATS], mmdt, tag="wkf")
        wv_full = singles.tile([128, KT, HG_FEATS], mmdt, tag="wvf")
        wo_sb = singles.tile([128, FT, D_MODEL], mmdt, tag="wo")

        # Initial loads split across HWDGE queues: sync carries what the
        # first V matmuls need (wv + stripe 0); vector carries wq/wk; scalar
        # carries wo (+ rope tables).
        nc.sync.dma_start(
            out=wv_full[:, 0 : KT // 2, :],
            in_=wvT.ap()[0 : D_MODEL // 2, :].rearrange("(k p) f -> p k f", p=128),
        )
        sh_first = stripes.tile([128, KT, 512], mmdt, tag="stripe", name="sh0")
        for w in range(2):
            ks = slice(w * KT // 2, (w + 1) * KT // 2)
            nc.sync.dma_start(
                out=sh_first[:, ks, :],
                in_=xT.ap()[(ks.start * 128) : (ks.stop * 128), 0:512].rearrange(
                    "(k p) s -> p k s", p=128
                ),
            )
        nc.sync.dma_start(
            out=wv_full[:, KT // 2 : KT, :],
            in_=wvT.ap()[D_MODEL // 2 :, :].rearrange("(k p) f -> p k f", p=128),
        )
        qldma = nc.scalar if K2_QSPLIT else nc.sync
        for wsb, wdr in ((wq_full, wqT), (wk_full, wkT)):
            qldma.dma_start(
                out=wsb, in_=wdr.ap().rearrange("(k p) f -> p k f", p=128)
            )
        if use_rope:
            cos_sb = singles.tile([128, s], f16, tag="cos")
            sin_sb = singles.tile([128, s], f16, tag="sin")
            qldma.dma_start(out=cos_sb, in_=cosT.ap())
            qldma.dma_start(out=sin_sb, in_=sinT.ap())
        _wo_load = [
            lambda: qldma.dma_start(
                out=wo_sb, in_=woT.ap().rearrange("(f p) o -> p f o", p=128)
            )
        ]

        # V resident in SBUF: per (kb, hp) cols [V_h0(64) | 1 | pad | V_h1(64) | 1 | pad]
        vres = singles.tile([128, ST, FT, 132], mmdt, tag="vres")
        nc.vector.memset(
            vres[:].rearrange("p st hp (h c) -> p (st hp h) c", c=66)[:, :, 64:65],
            1.0,
        )
        # Z rows roundtrip through DRAM: [1, 1024] -> [64, 16] for a 64-lane
        # reciprocal, then 1/Z broadcast back across 64 partitions.
        ztmp = dramp.tile([FT, SC, 1, 1024], f32, tag="ztmp")
        ztmp2 = dramp.tile([FT, SC, 1, 1024], f16, tag="ztmp2")
        QTs = [
            singles.tile([128, s], mmdt, tag=f"QT{hp}", name=f"QT{hp}")
            for hp in range(FT)
        ]
        KTs = [
            singles.tile([128, s], mmdt, tag=f"KT{hp}", name=f"KT{hp}")
            for hp in range(FT)
        ]

        def load_stripe(sc):
            sh = stripes.tile([128, KT, 512], mmdt, tag="stripe", name=f"sh{sc}")
            nc.sync.dma_start(
                out=sh,
                in_=xT.ap()[:, sc * 512 : (sc + 1) * 512].rearrange(
                    "(k p) s -> p k s", p=128
                ),
            )
            return sh

        # ---- work chains (each closure emits ~1-2us of PE work) ----------
        def vproj_chain(sc, sh, j):
            def emit():
                st = sc * 4 + j
                pv = psum.tile([128, 512], f32, tag="pqk", bufs=2, name="pv")
                for kt in range(KT):
                    nc.tensor.matmul(
                        pv,
                        lhsT=sh[:, kt, j * 128 : (j + 1) * 128],
                        rhs=wv_full[:, kt, :],
                        start=(kt == 0),
                        stop=(kt == KT - 1),
                    )
                # scatter into vres: dest inner 64 f16 contiguous, 132-col
                # blocks keep h-offsets 4B-aligned (DVE evacuation)
                nc.vector.tensor_copy(
                    vres[:, st].rearrange("p hp (h c) -> p (hp h) c", c=66)[
                        :, :, 0:64
                    ],
                    pv[:].rearrange("p (x c) -> p x c", c=64),
                )
            return emit

        def qkproj_chain(sc, sh, hp, which):
            def emit():
                w_sb = wq_full if which == 0 else wk_full
                dst = QTs[hp] if which == 0 else KTs[hp]
                pq = psum.tile([128, 512], f32, tag="pqk", bufs=2, name="pq")
                for kt in range(KT):
                    nc.tensor.matmul(
                        pq,
                        lhsT=w_sb[:, kt, hp * 128 : (hp + 1) * 128],
                        rhs=sh[:, kt, :],
                        start=(kt == 0),
                        stop=(kt == KT - 1),
                    )
                dcol = dst[:, sc * 512 : (sc + 1) * 512]
                if use_rope:
                    ccol = cos_sb[:, sc * 512 : (sc + 1) * 512]
                    scol = sin_sb[:, sc * 512 : (sc + 1) * 512]
                    qsb = tmppool.tile([128, 512], f16, tag="qsb")
                    qcos = tmppool.tile([128, 512], f16, tag="qcos")
                    rot = tmppool.tile([128, 512], f16, tag="rot")
                    nc.scalar.activation(qsb, pq, FP.Copy)
                    nc.vector.stream_shuffle(rot, qsb, PAIRSWAP)
                    nc.vector.tensor_mul(qcos, qsb, ccol)
                    nc.vector.tensor_mul(rot, rot, scol)
                    nc.vector.tensor_add(dcol, qcos, rot)
                else:
                    nc.vector.tensor_copy(dcol, pq)
            return emit

        def wo_chain_base(qc, ctxn, j):
            def emit():
                st = qc * 4 + j
                osb = outpool.tile([128, 1024], f32, tag="osb")
                for half in range(2):
                    po = [
                        psum.tile([128, 512], f32, tag="pqk", bufs=2, name=f"po{h}")
                        for h in range(2)
                    ]
                    for hp in range(FT):
                        for h in range(2):
                            nc.tensor.matmul(
                                po[h],
                                lhsT=ctxn[hp][
                                    64 * h : 64 * h + 64, j * 128 : (j + 1) * 128
                                ],
                                rhs=wo_sb[
                                    64 * h : 64 * h + 64,
                                    hp,
                                    half * 512 : (half + 1) * 512,
                                ],
                                start=(hp == 0),
                                stop=(hp == FT - 1),
                                skip_group_check=True,
                            )
                    oh = osb[:, half * 512 : (half + 1) * 512]
                    nc.scalar.activation(oh, po[0], FP.Copy)
                    nc.vector.scalar_tensor_tensor(
                        out=oh,
                        in0=po[1],
                        scalar=1.0,
                        in1=oh,
                        op0=mybir.AluOpType.mult,
                        op1=mybir.AluOpType.add,
                    )
                nc.sync.dma_start(
                    out=out.ap()[st * 128 : (st + 1) * 128, :], in_=osb
                )
            return emit

        def wo_chain(qc, ctxn, j, epilogue=False):
            if not K2_WOPAIR:
                return wo_chain_base(qc, ctxn, j)

            def emit():
                st = qc * 4 + j
                # 4 chains, each with a consistent PE row group:
                #   h0 chains (rows 0-63)  -> osb_a -> out_a
                #   h1 chains (rows 64-127) -> osb_b -> out_b
                # (h0, half) and (h1, half) emitted adjacently -> concurrent.
                osb_a = outpool.tile([128, 1024], f16, tag="osba")
                osb_b = outpool.tile([128, 1024], f16, tag="osbb")
                for half in range(2):
                    if epilogue and (2 * j + half) % 2 == 1:
                        pop = psum.tile(
                            [128, 1024], f32, tag="oacc", bufs=1, name="pop"
                        )
                        po0 = pop[:, 0:512]
                        po1 = pop[:, 512:1024]
                    else:
                        po0 = psum.tile([128, 512], f32, tag="pqk", bufs=2, name="po0")
                        po1 = psum.tile([128, 512], f32, tag="pqk", bufs=2, name="po1")
                    for hp in range(FT):
                        for h, po in ((0, po0), (1, po1)):
                            nc.tensor.matmul(
                                po,
                                lhsT=ctxn[hp][
                                    64 * h : 64 * h + 64, j * 128 : (j + 1) * 128
                                ],
                                rhs=wo_sb[
                                    64 * h : 64 * h + 64,
                                    hp,
                                    half * 512 : (half + 1) * 512,
                                ],
                                start=(hp == 0),
                                stop=(hp == FT - 1),
                                skip_group_check=True,
                            )
                    # evacuate: one on ScalarE, one on DVE (engine balance)
                    nc.scalar.activation(
                        osb_a[:, half * 512 : (half + 1) * 512], po0, FP.Copy
                    )
                    nc.vector.tensor_copy(
                        osb_b[:, half * 512 : (half + 1) * 512], po1
                    )
                nc.sync.dma_start(
                    out=out_a.ap()[st * 128 : (st + 1) * 128, :], in_=osb_a
                )
                nc.sync.dma_start(
                    out=out_b.ap()[st * 128 : (st + 1) * 128, :], in_=osb_b
                )
            return emit

        # ---- attention ---------------------------------------------------
        def do_attn(qc, work):
            nkb = 4 * qc + 4
            ctxn = [
                ctxpool.tile([128, 512], f16, tag=f"ctxn{hp}", name=f"ctxn{hp}")
                for hp in range(FT)
            ]
            for hp in range(FT):
                QT = QTs[hp]
                KTt = KTs[hp]
                oacc = psum.tile([128, 1024], f32, tag="oacc", bufs=1, name="oacc")
                def emit_attnv(kb, expt):
                    j = kb - 4 * qc
                    q0 = 128 * j if (j >= 0 and K2_TRIM) else 0
                    for h in range(2):
                        nc.tensor.matmul(
                            oacc[0:65, 512 * h + q0 : 512 * h + 512],
                            lhsT=vres[:, kb, hp, 66 * h : 66 * h + 65],
                            rhs=expt[:, 512 * h + q0 : 512 * h + 512],
                            start=(kb == 0),
                            stop=(kb == nkb - 1),
                            skip_group_check=True,
                        )

                prev = None
                for kb in range(nkb):
                    if kb % 2 == 0:
                        _drain_due(work, (qc, hp, kb // 2))
                    # scores [128, 1024]: [h0 | h1], double-buffered;
                    # diagonal blocks trim fully-masked query columns
                    j = kb - 4 * qc
                    q0 = 128 * j if (j >= 0 and K2_TRIM) else 0
                    scr = psum.tile(
                        [128, 1024], f32, tag="scores", bufs=2, name="scores"
                    )
                    for h in range(2):
                        nc.tensor.matmul(
                            scr[:, 512 * h + q0 : 512 * h + 512],
                            lhsT=KTt[
                                64 * h : 64 * h + 64, kb * 128 : (kb + 1) * 128
                            ],
                            rhs=QT[
                                64 * h : 64 * h + 64,
                                qc * 512 + q0 : (qc + 1) * 512,
                            ],
                            start=True,
                            stop=True,
                            skip_group_check=True,
                        )
                    expt = exppool.tile([128, 1024], mmdt, tag="expt")
                    if q0 == 0:
                        nc.scalar.activation(expt, scr, FP.Exp, scale=0.125)
                    else:
                        for h in range(2):
                            nc.scalar.activation(
                                expt[:, 512 * h + q0 : 512 * h + 512],
                                scr[:, 512 * h + q0 : 512 * h + 512],
                                FP.Exp,
                                scale=0.125,
                            )
                    if j >= 0:  # diagonal tile: causal mask post-exp
                        q0 = 128 * j if K2_TRIM else 0
                        sl = expt.rearrange("p (h q) -> p h q", h=2)[:, :, q0:512]
                        nc.gpsimd.affine_select(
                            out=sl,
                            in_=sl,
                            compare_op=mybir.AluOpType.is_ge,
                            fill=0.0,
                            base=-(128 * j - q0),
                            pattern=[[0, 2], [1, 512 - q0]],
                            channel_multiplier=-1,
                        )
                    # previous kb's attn@V (exp/mask latency covered)
                    if prev is not None:
                        emit_attnv(*prev)
                    prev = (kb, expt)
                emit_attnv(*prev)
                # ---- evacuate ctx+Z, normalize -------------------------
                ctxu = smallp.tile([128, 1024], f32, tag="ctxu")
                nc.vector.tensor_copy(ctxu[0:65, :], oacc[0:65, :])
                dmae = nc.sync
                dmae.dma_start(out=ztmp[hp, qc], in_=ctxu[64:65, :])
                zT = smallp.tile([64, 16], f32, tag="zT")
                dmae.dma_start(
                    out=zT, in_=ztmp[hp, qc, 0].rearrange("(p g) -> p g", p=64)
                )
                zinvT = smallp.tile([64, 16], f32, tag="zinvT")
                nc.vector.reciprocal(zinvT, zT)
                zinvT16 = smallp.tile([64, 16], f16, tag="zinvT16")
                nc.vector.tensor_copy(zinvT16, zinvT)
                dmae.dma_start(
                    out=ztmp2[hp, qc, 0].rearrange("(p g) -> p g", p=64),
                    in_=zinvT16,
                )
                zbc_sb = smallp.tile([128, 1024], f16, tag="zbcsb")
                dmae.dma_start(
                    out=zbc_sb[0:64, :],
                    in_=ztmp2[hp, qc].broadcast_to([64, 1024]),
                )
                for h in range(2):
                    nc.vector.scalar_tensor_tensor(
                        out=ctxn[hp][64 * h : 64 * h + 64, :],
                        in0=ctxu[0:64, 512 * h : 512 * h + 512],
                        scalar=1.0,
                        in1=zbc_sb[0:64, 512 * h : 512 * h + 512],
                        op0=mybir.AluOpType.mult,
                        op1=mybir.AluOpType.mult,
                    )
            return ctxn

        # ---- deadline-scheduled emission ---------------------------------
        # Every chain gets an emission deadline (qc, hp, kbp): it is emitted
        # just before that attention step.  Deadlines place each chain where
        # the PE needs fill (late chunks have more exp time to cover), always
        # at-or-before the step that first reads the chain's outputs.
        work = []

        def _drain_due(work, step):
            i = 0
            while i < len(work):
                dl, em = work[i]
                if dl <= step:
                    em()
                    work.pop(i)
                else:
                    i += 1

        # proj(0): V j0/j1 + QK hp0 land in the prologue (deadline (0,0,0));
        # the rest weave into attn(0) at their first-read step.
        PROJ_DL = {
            0: {("v", 0): (0, 0, 0), ("v", 1): (0, 0, 0),
                ("v", 2): (0, 0, 1), ("v", 3): (0, 0, 1),
                ("qk", 0, 0): (0, 0, 0), ("qk", 0, 1): (0, 0, 0),
                ("qk", 1, 0): (0, 1, 0), ("qk", 1, 1): (0, 1, 0),
                ("qk", 2, 0): (0, 2, 0), ("qk", 2, 1): (0, 2, 0),
                ("qk", 3, 0): (0, 3, 0), ("qk", 3, 1): (0, 3, 0)},
            # proj(1): V + QK(hp0) inside attn(0); the rest early in attn(1)
            1: {("v", 0): (0, 0, 1), ("v", 1): (0, 1, 1),
                ("v", 2): (0, 2, 1), ("v", 3): (0, 3, 1),
                ("qk", 0, 0): (0, 1, 1), ("qk", 0, 1): (0, 2, 1),
                ("qk", 1, 0): (1, 0, 0), ("qk", 1, 1): (1, 0, 1),
                ("qk", 2, 0): (1, 1, 0), ("qk", 2, 1): (1, 1, 1),
                ("qk", 3, 0): (1, 2, 0), ("qk", 3, 1): (1, 2, 1)},
            # proj(2): spread across attn(1)
            2: {("v", 0): (1, 0, 1), ("v", 1): (1, 1, 1),
                ("v", 2): (1, 2, 1), ("v", 3): (1, 3, 1),
                ("qk", 0, 0): (1, 0, 2), ("qk", 0, 1): (1, 0, 3),
                ("qk", 1, 0): (1, 1, 2), ("qk", 1, 1): (1, 1, 3),
                ("qk", 2, 0): (1, 2, 2), ("qk", 2, 1): (1, 2, 3),
                ("qk", 3, 0): (1, 3, 2), ("qk", 3, 1): (1, 3, 3)},
            # proj(3): V + QK(hp0) spread across attn(2); QK(hp1-3) spill
            # into attn(3) just before each hp needs them
            3: {("v", 0): (2, 0, 2), ("v", 1): (2, 0, 5),
                ("v", 2): (2, 1, 1), ("v", 3): (2, 1, 4),
                ("qk", 0, 0): (2, 0, 0), ("qk", 0, 1): (2, 2, 0),
                ("qk", 1, 0): (3, 0, 1), ("qk", 1, 1): (3, 0, 4),
                ("qk", 2, 0): (3, 1, 1), ("qk", 2, 1): (3, 1, 5),
                ("qk", 3, 0): (3, 2, 2), ("qk", 3, 1): (3, 2, 4)},
        }
        # wo(sc) spread: wo(0) into attn(2); wo(1)/wo(2) into attn(3)
        WO_DL = {
            0: [(2, 2, 2), (2, 2, 5), (2, 3, 1), (2, 3, 3)],
            1: [(3, 0, 6), (3, 1, 3), (3, 1, 7), (3, 2, 6)],
            # wo(2): no in-loop deadline -> emitted at the post-loop flush,
            # leaving ready PE work to cover the last Z-chain latency
            2: [(3, 99, 0), (3, 99, 0), (3, 99, 0), (3, 99, 0)],
        }

        def add_proj_work(sc, sh):
            dls = PROJ_DL[sc]
            for j in range(4):
                work.append((dls[("v", j)], vproj_chain(sc, sh, j)))
            for hp in range(FT):
                for w in (0, 1):
                    work.append((dls[("qk", hp, w)], qkproj_chain(sc, sh, hp, w)))

        def warmfill():
            # HAM warm-keeper: dummy MMs gated on the last wv piece so they
            # fill the early DMA-starvation gap; results are overwritten by
            # the first real scores (start=True clears the banks).
            scr = psum.tile([128, 1024], f32, tag="scores", bufs=2, name="warm")
            for i in range(8):
                nc.tensor.matmul(
                    scr[:, 512 * (i % 2) : 512 * (i % 2) + 512],
                    lhsT=wv_full[:, KT - 1, (i % 4) * 128 : (i % 4) * 128 + 128],
                    rhs=wv_full[:, KT - 2, 0:512],
                    start=True,
                    stop=True,
                    skip_group_check=True,
                )

        add_proj_work(0, sh_first)
        work.append(((0, 0, 0), warmfill))
        work.append(((0, 2, 0), _wo_load[0]))
        ctxs = {}
        for sc in range(SC):
            if sc + 1 < SC:
                sh = load_stripe(sc + 1)
                add_proj_work(sc + 1, sh)
            if sc - 1 >= 0 and sc - 1 in WO_DL:
                prev_ctx = ctxs[sc - 1]
                for j in range(4):
                    work.append((WO_DL[sc - 1][j], wo_chain(sc - 1, prev_ctx, j)))
            ctxs[sc] = do_attn(sc, work)
        for dl, em in work:  # anything left (defensive)
            em()
        work.clear()
        last_ctx = ctxs[SC - 1]
        for em in [wo_chain(SC - 1, last_ctx, j, epilogue=True) for j in range(4)]:
            em()

    nc.compile()
    return nc


def _round_f32r(a):
    """Round fp32 array to the PE's FP32R format (RNE at 12 low mantissa bits)."""
    u = np.ascontiguousarray(a, np.float32).view(np.uint32).astype(np.uint64)
    low = u & 0xFFF
    up = (low > 0x800) | ((low == 0x800) & (((u >> 12) & 1) == 1))
    r = (u & ~np.uint64(0xFFF)) + np.where(up, 0x1000, 0)
    return r.astype(np.uint32).view(np.float32)


def _to_f16(a):
    return np.ascontiguousarray(a, np.float16)


def _rope_tables(s: int):
    inv_freq = 1.0 / (ROPE_THETA ** (np.arange(0, D_K, 2, dtype=np.float64) / D_K))
    angles = np.arange(s, dtype=np.float64)[:, None] * inv_freq[None, :]  # [s, 32]
    cos = np.cos(angles).astype(np.float32)  # [s, 32]
    sin = np.sin(angles).astype(np.float32)
    cosT = np.empty((D_K, s), np.float32)
    sinT = np.empty((D_K, s), np.float32)
    cosT[0::2] = cos.T
    cosT[1::2] = cos.T
    sinT[0::2] = -sin.T
    sinT[1::2] = sin.T
    return (
        np.ascontiguousarray(np.vstack([cosT, cosT])).astype(np.float16),
        np.ascontiguousarray(np.vstack([sinT, sinT])).astype(np.float16),
    )


def kernel(x, Wq, Wk, Wv, Wo, use_rope):
    from concourse.bass_utils import run_bass_kernel_spmd

    x = np.asarray(x, dtype=np.float32)
    ur = bool(int(np.asarray(use_rope)))
    key = (ur, S)
    if key not in _PROGRAM_CACHE:
        _PROGRAM_CACHE[key] = _build_program(ur, S)
    nc = _PROGRAM_CACHE[key]

    if ur:
        cosT, sinT = _rope_tables(S)

    in_maps = []
    for c in range(N_CORES):
        b, hg = c // 2, c % 2
        sl = slice(hg * HG_FEATS, (hg + 1) * HG_FEATS)
        cv = _round_f32r if MM_DTYPE == "f32r" else _to_f16
        m = {
            "xT": cv(x[b].T),
            "wqT": cv(np.asarray(Wq, np.float32)[sl, :].T),
            "wkT": cv(np.asarray(Wk, np.float32)[sl, :].T),
            "wvT": cv(np.asarray(Wv, np.float32)[sl, :].T),
            "woT": cv(np.asarray(Wo, np.float32)[:, sl].T),
        }
        if ur:
            m["cosT"] = cosT
            m["sinT"] = sinT
        in_maps.append(m)

    res = run_bass_kernel_spmd(nc, in_maps, list(range(N_CORES)))
    out = np.empty((B, S, D_MODEL), np.float32)
    for b in range(B):
        if K2_WOPAIR:
            out[b] = (
                res.results[2 * b]["out_a"].astype(np.float32)
                + res.results[2 * b]["out_b"].astype(np.float32)
                + res.results[2 * b + 1]["out_a"].astype(np.float32)
                + res.results[2 * b + 1]["out_b"].astype(np.float32)
            )
        else:
            out[b] = res.results[2 * b]["out"] + res.results[2 * b + 1]["out"]
    return out
